# revision 22
# baseline (speedup 1.0000x reference)
"""AdderNet layer (adder2d + residual + BatchNorm(train) + PowerActivation)
on 8 Trainium2 NeuronCores. Raw Bass, explicit semaphores.

v4: all-abs production via the abs_max ALU trick (|x-w| = abs_max(x-w, 0)
in ONE DVE tensor_scalar at the 4x perf mode), giving every tap tile a
uniform PE coefficient of -1. This removes the v3 box-sum passes and the
min/abs algebra split. 32-row tiles (2x fewer elementwise ops than v3's
16-row groups -> amortized per-op overhead), PSUM laid out as
[16 outs, 8 banks] per 32-row pass (two pass buffers at partition bases
0/32 of one [48,8,512] tensor), so evac is ONE activation per pass, and
Sum(y^2) is accumulated during evac (kills the separate variance pass
and the mean64 broadcast). The fp16 y buffer lives in DRAM (frees
~30KB/partition of SBUF); the BN affine tail streams it back through
the freed tmp buffers.

Per (j, 32-row pass): 6 DVE abs tiles + 3 ACT abs tiles; DVE pre-merges
the vertical pairs (kh0,kw)+(kh1,kw) for kw=0,1 -> 7 PE clusters of 8
matmuls each + shared residual. Engine busy model per pass: DVE ~95us,
ACT ~97us, PE ~97us - balanced within ~3%.

Self-contained: hardcodes N,C,H,W=8,64,128,128, CO=64, K=3, pad=1.
Sharding by OUTPUT CHANNEL (8 co per core): BN stats core-local.
PowerActivation alpha=1.0 is identity (harness value); exact host
fallback otherwise. kernel() runs a warmup execution first.
"""

import math
import os
from contextlib import ExitStack

import numpy as np

N, C, H, W = 8, 64, 128, 128
CO, KS = 64, 3
BN_EPS = 1e-5
NCORES = 8
CP = CO // NCORES
RW = 132
ROWS = 66
PIX = H * W
CNT = float(N * PIX)
NPASS = 2 * N            # 16 32-row passes

# ring sizes (tiles of [128, 32, <=130] fp16, ~8.3 KB/partition)
RD_N = 4    # DVE pair-feed tiles (consumed by DVE merges, same queue)
RP_N = 4    # merged pair tiles -> PE
RS_N = 4    # DVE single tiles (kw=2) -> PE
RA_N = 4    # ACT abs tiles -> PE

SEL_PAIR0 = 0             # selmm slices: 0..7 pair/min lhs (+2) per j
SEL_ABS0 = 8              # 8..15 abs lhs (-1) per j
SEL_BOX = 16              # -1 box-sum, all j columns
SEL_RES = 17              # +1 residual
NSEL = 18

COL_G = 72
COL_B = 73
COL_OFF = 74
COL_S1F = 76              # sel: [2j+b, j] = 1 (rows 0:16, 8 cols)
NC32 = 92

TS_PER_PASS = 6 * CP      # DVE tensor_scalar productions per 32-row pass
TA_PER_PASS = 3 * CP      # ACT productions per pass


def _build_program():
    import concourse.bass as bass
    import concourse.mybir as mybir
    from concourse.mybir import AluOpType as Op

    f32 = mybir.dt.float32
    f16 = mybir.dt.float16
    AF = mybir.ActivationFunctionType

    nc = bass.Bass("TRN2")

    x16p = nc.dram_tensor("x16p", [N, 128, ROWS * RW], f16,
                          kind="ExternalInput")
    consts32 = nc.dram_tensor("consts32", [128, NC32], f32,
                              kind="ExternalInput")
    selmm = nc.dram_tensor("selmm", [128, NSEL, 16], f16,
                           kind="ExternalInput")
    out = nc.dram_tensor("out", [64, PIX], f16, kind="ExternalOutput")
    ybuf = nc.dram_tensor("ybuf", [16, NPASS, 4096], f16, kind="Internal")
    bnscr = nc.dram_tensor("bnscr", [1, 16], f32, kind="Internal")

    ctx = ExitStack()
    with ctx:
        c32 = ctx.enter_context(nc.sbuf_tensor("c32", [128, NC32], f32))
        selmm_sb = ctx.enter_context(
            nc.sbuf_tensor("selmm_sb", [128, NSEL, 16], f16))
        xpad0 = ctx.enter_context(nc.sbuf_tensor("xpad0", [128, ROWS, RW], f16))
        xpad1 = ctx.enter_context(nc.sbuf_tensor("xpad1", [128, ROWS, RW], f16))
        xpads = [xpad0, xpad1]
        RD = [ctx.enter_context(nc.sbuf_tensor(f"RD{i}", [128, 32, 130], f16))
              for i in range(RD_N)]
        RP = [ctx.enter_context(nc.sbuf_tensor(f"RP{i}", [128, 32, 130], f16))
              for i in range(RP_N)]
        RS = [ctx.enter_context(nc.sbuf_tensor(f"RS{i}", [128, 32, 128], f16))
              for i in range(RS_N)]
        RA = [ctx.enter_context(nc.sbuf_tensor(f"RA{i}", [128, 32, 128], f16))
              for i in range(RA_N)]
        xvs = [ctx.enter_context(nc.sbuf_tensor(f"xv{i}", [128, 32, 131], f16))
               for i in range(2)]
        tmp0 = ctx.enter_context(nc.sbuf_tensor("tmp0", [16, 4096], f16))
        tmp1 = ctx.enter_context(nc.sbuf_tensor("tmp1", [16, 4096], f16))
        tmps = [tmp0, tmp1]
        ysq = ctx.enter_context(nc.sbuf_tensor("ysq", [16, 4096], f16))
        s1cols = ctx.enter_context(nc.sbuf_tensor("s1cols", [16, NPASS], f32))
        s2cols = ctx.enter_context(nc.sbuf_tensor("s2cols", [16, NPASS], f32))
        s1p = ctx.enter_context(nc.sbuf_tensor("s1p", [16, 1], f32))
        s2p = ctx.enter_context(nc.sbuf_tensor("s2p", [16, 1], f32))
        mean8 = ctx.enter_context(nc.sbuf_tensor("mean8", [8, 1], f32))
        ey28 = ctx.enter_context(nc.sbuf_tensor("ey28", [8, 1], f32))
        msq = ctx.enter_context(nc.sbuf_tensor("msq", [8, 1], f32))
        var8 = ctx.enter_context(nc.sbuf_tensor("var8", [8, 1], f32))
        sqt = ctx.enter_context(nc.sbuf_tensor("sqt", [8, 1], f32))
        rt = ctx.enter_context(nc.sbuf_tensor("rt", [8, 1], f32))
        ut = ctx.enter_context(nc.sbuf_tensor("ut", [8, 1], f32))
        scsh8 = ctx.enter_context(nc.sbuf_tensor("scsh8", [8, 2], f32))
        AB16 = ctx.enter_context(nc.sbuf_tensor("AB16", [16, 2], f32))

        acc48 = ctx.enter_context(nc.psum_tensor("acc48", [48, 8, 512], f32))

        s_dmac = ctx.enter_context(nc.semaphore())
        s_dmax0 = ctx.enter_context(nc.semaphore())
        s_dmax1 = ctx.enter_context(nc.semaphore())
        s_dmaxs = [s_dmax0, s_dmax1]
        s_Td = ctx.enter_context(nc.semaphore())   # DVE TS productions
        s_Ta = ctx.enter_context(nc.semaphore())   # ACT productions
        s_pr = ctx.enter_context(nc.semaphore())   # DVE pair merges
        s_pg = ctx.enter_context(nc.semaphore())   # PE consumed pair
        s_ms = ctx.enter_context(nc.semaphore())   # PE consumed RS single
        s_sg = ctx.enter_context(nc.semaphore())   # PE consumed ACT tile
        s_ev = ctx.enter_context(nc.semaphore())   # evac done (per pass)
        s_sq = ctx.enter_context(nc.semaphore())   # square done (per pass)
        s_ev2 = ctx.enter_context(nc.semaphore())  # PE pass drain
        s_ydma = ctx.enter_context(nc.semaphore())  # ybuf store done
        s_pe = ctx.enter_context(nc.semaphore())   # PE BN folds
        s_dv = ctx.enter_context(nc.semaphore())   # DVE BN steps
        s_ac = ctx.enter_context(nc.semaphore())   # ACT BN steps
        s_vc = ctx.enter_context(nc.semaphore())   # DVE serial chain
        s_bn = ctx.enter_context(nc.semaphore())   # BN dma chain
        s_yin = ctx.enter_context(nc.semaphore())  # affine chunk loads
        s_p3 = ctx.enter_context(nc.semaphore())   # affine chunks done
        s_xv = ctx.enter_context(nc.semaphore())   # xv tile produced
        s_xvc = ctx.enter_context(nc.semaphore())  # PE consumed xv
        block = ctx.enter_context(nc.Block())

        gma = c32[0:8, COL_G:COL_G + 1]
        bta = c32[0:8, COL_B:COL_B + 1]
        cOFF = c32[0:16, COL_OFF:COL_OFF + 1]

        def wcol(j, t):
            return c32[:, j * 9 + t:j * 9 + t + 1]

        def src(n, h, kh):
            base = 32 * h + kh
            return xpads[n % 2][:, base:base + 32, :]

        def abase(P):
            return 32 * (P % 2)

        # global stage index: stage = P * 8 + j, P = 2n + h
        # ---------------- Pool: DMA loads ----------------
        @block.gpsimd
        def _(gp):
            gp.dma_start(c32[:], consts32[:]).then_inc(s_dmac, 16)
            gp.dma_start(selmm_sb[:], selmm[:]).then_inc(s_dmac, 16)
            for n in range(2):
                gp.dma_start(
                    xpads[n][:].rearrange("p r c -> p (r c)"),
                    x16p[n, :, :]).then_inc(s_dmaxs[n], 16)
            for nl in range(2, N):
                # image nl-2's readers must be done before overwrite
                gp.wait_ge(s_ev2, 2 * (nl - 1))
                gp.wait_ge(s_Td, TS_PER_PASS * 2 * (nl - 1))
                gp.wait_ge(s_Ta, TA_PER_PASS * 2 * (nl - 1))
                gp.dma_start(
                    xpads[nl % 2][:].rearrange("p r c -> p (r c)"),
                    x16p[nl, :, :]).then_inc(s_dmaxs[nl % 2], 16)

        # ---------------- DVE: abs tiles + pair merges + BN ----------
        @block.vector
        def _(v_):
            v_.wait_ge(s_dmac, 32)
            for n in range(N):
                for h in range(2):
                    P = 2 * n + h
                    if h == 0:
                        v_.wait_ge(s_dmaxs[n % 2], 16 * (n // 2 + 1))
                    # xv: row-pair sum for the box passes
                    if P >= 2:
                        v_.wait_ge(s_xvc, 3 * (P - 1))
                    v_.tensor_tensor(
                        xvs[P % 2][:, :, 0:131],
                        src(n, h, 0)[:, :, 0:131],
                        src(n, h, 1)[:, :, 0:131],
                        Op.add).then_inc(s_xv, 1)
                    for j in range(CP):
                        st = P * 8 + j
                        # pair feeds: taps (0,0),(0,1),(1,0),(1,1)
                        for mi, (kh, kw) in enumerate(
                                ((0, 0), (0, 1), (1, 0), (1, 1))):
                            fidx = st * 4 + mi
                            v_.tensor_scalar(
                                RD[fidx % RD_N][:, :, 0:130],
                                src(n, h, kh)[:, :, 0:130],
                                wcol(j, 3 * kh + kw), 0.0,
                                Op.subtract, Op.min).then_inc(s_Td, 1)
                        # singles: taps (0,2),(1,2) — emitted between the
                        # feeds and the merges so the merge reads are >2us
                        # behind the feed writes (same-engine RAW pipeline)
                        for i in range(2):
                            sidx = st * 2 + i
                            if sidx - (RS_N - 1) > 0:
                                v_.wait_ge(s_ms, sidx - (RS_N - 1))
                            v_.tensor_scalar(
                                RS[sidx % RS_N][:, :, 0:128],
                                src(n, h, i)[:, :, 2:130],
                                wcol(j, 3 * i + 2), 0.0,
                                Op.subtract, Op.min).then_inc(s_Td, 1)
                        for k in range(2):  # merge (0,k)+(1,k)
                            pidx = st * 2 + k
                            if pidx - (RP_N - 1) > 0:
                                v_.wait_ge(s_pg, pidx - (RP_N - 1))
                            fa = st * 4 + k
                            fb = st * 4 + 2 + k
                            v_.tensor_tensor(
                                RP[pidx % RP_N][:, :, 0:130],
                                RD[fa % RD_N][:, :, 0:130],
                                RD[fb % RD_N][:, :, 0:130],
                                Op.add).then_inc(s_pr, 1)

            # ---- BN tail ----
            v_.wait_ge(s_ev, NPASS)
            v_.tensor_reduce(s1p[:], s1cols[:], mybir.AxisListType.X,
                             Op.add).then_inc(s_dv, 1)
            v_.wait_ge(s_sq, NPASS)
            v_.tensor_reduce(s2p[:], s2cols[:], mybir.AxisListType.X,
                             Op.add).then_inc(s_dv, 1)
            v_.wait_ge(s_ac, 2)
            vcnt = 0

            def vstep(inst):
                nonlocal vcnt
                vcnt += 1
                inst.then_inc(s_vc, 1)
                v_.wait_ge(s_vc, vcnt)

            vstep(v_.tensor_tensor(msq[:], mean8[:], mean8[:], Op.mult))
            vstep(v_.tensor_tensor(var8[:], ey28[:], msq[:], Op.subtract))
            v_.tensor_scalar_add(var8[:], var8[:], BN_EPS).then_inc(s_dv, 1)
            v_.wait_ge(s_ac, 3)
            vstep(v_.reciprocal(rt[:], sqt[:]))
            for _i in range(2):
                vstep(v_.tensor_tensor(ut[:], rt[:], rt[:], Op.mult))
                vstep(v_.tensor_tensor(ut[:], ut[:], var8[:], Op.mult))
                vstep(v_.tensor_scalar(ut[:], ut[:], -0.5, 1.5,
                                       Op.mult, Op.add))
                vstep(v_.tensor_tensor(rt[:], rt[:], ut[:], Op.mult))
            vstep(v_.tensor_tensor(scsh8[:, 0:1], gma, rt[:], Op.mult))
            vstep(v_.tensor_tensor(scsh8[:, 1:2], mean8[:], scsh8[:, 0:1],
                                   Op.mult))
            v_.tensor_tensor(scsh8[:, 1:2], bta, scsh8[:, 1:2],
                             Op.subtract).then_inc(s_dv, 1)
            # affine chunks (one pass each) in the freed tmp buffers
            v_.wait_ge(s_bn, 32)
            for cch in range(NPASS):
                v_.wait_ge(s_yin, 16 * (cch + 1))
                v_.tensor_scalar(
                    tmps[cch % 2][:], tmps[cch % 2][:],
                    AB16[:, 0:1], AB16[:, 1:2],
                    Op.mult, Op.add).then_inc(s_p3, 1)

        # ---------------- PE: reduction matmuls ----------------
        @block.tensor
        def _(t_):
            t_.wait_ge(s_dmac, 32)
            for n in range(N):
                for h in range(2):
                    P = 2 * n + h
                    ab = abase(P)
                    if P >= 2:
                        t_.wait_ge(s_ev, P - 1)
                    if h == 0:
                        t_.wait_ge(s_dmaxs[n % 2], 16 * (n // 2 + 1))
                    # residual: start=True zeroes this pass's banks
                    for rq in range(2):
                        for cb in range(4):
                            t_.matmul(
                                acc48[ab:ab + 16, 4 * rq + cb, :],
                                selmm_sb[:, SEL_RES, :],
                                xpads[n % 2][:, 1 + 32 * h + 16 * rq + 4 * cb:
                                             1 + 32 * h + 16 * rq + 4 * cb + 4,
                                             1:129],
                                start=True, stop=False,
                                skip_group_check=True)

                    def unit(tile, sel, colsl, sem, is_last):
                        for rq in range(2):
                            for cb in range(4):
                                mm = t_.matmul(
                                    acc48[ab:ab + 16, 4 * rq + cb, :],
                                    selmm_sb[:, sel, :],
                                    tile[:, 16 * rq + 4 * cb:
                                         16 * rq + 4 * cb + 4, colsl],
                                    start=False,
                                    stop=is_last and rq == 1 and cb == 3,
                                    skip_group_check=True)
                                if rq == 1 and cb == 3:
                                    mm.then_inc(sem, 1)

                    # box passes: -x over the 6 min taps via xv
                    t_.wait_ge(s_xv, P + 1)
                    unit(xvs[P % 2], SEL_BOX, slice(0, 128), s_xvc, False)
                    unit(xvs[P % 2], SEL_BOX, slice(1, 129), s_xvc, False)
                    unit(xvs[P % 2], SEL_BOX, slice(2, 130), s_xvc, False)
                    for j in range(CP):
                        st = P * 8 + j
                        last_j = (j == CP - 1)
                        for k in range(2):  # pairs
                            pidx = st * 2 + k
                            t_.wait_ge(s_pr, pidx + 1)
                            unit(RP[pidx % RP_N], SEL_PAIR0 + j,
                                 slice(k, k + 128), s_pg, False)
                        for i in range(2):  # DVE singles
                            t_.wait_ge(s_Td, st * 6 + 5 + i)
                            sidx = st * 2 + i
                            unit(RS[sidx % RS_N], SEL_PAIR0 + j,
                                 slice(0, 128), s_ms, False)
                        for k in range(3):  # ACT tiles
                            aidx = st * 3 + k
                            t_.wait_ge(s_Ta, aidx + 1)
                            unit(RA[aidx % RA_N], SEL_ABS0 + j,
                                 slice(0, 128), s_sg, last_j and k == 2)
                    t_.drain().then_inc(s_ev2, 1)
            # BN folds: [16] -> [8] selection matmuls
            t_.wait_ge(s_dv, 1)
            t_.matmul(acc48[0:8, 0, 0:1], c32[0:16, COL_S1F:COL_S1F + 8],
                      s1p[:], start=True, stop=True,
                      skip_group_check=True).then_inc(s_pe, 1)
            t_.wait_ge(s_dv, 2)
            t_.matmul(acc48[0:8, 1, 0:1], c32[0:16, COL_S1F:COL_S1F + 8],
                      s2p[:], start=True, stop=True,
                      skip_group_check=True).then_inc(s_pe, 1)

        # ---------------- ACT: abs tiles + evac + BN tail -------------
        @block.scalar
        def _(a):
            a.wait_ge(s_dmac, 32)

            def evac(P):
                # pass P: wait PE drain, evacuate acc -> tmp, square, store
                a.wait_ge(s_ev2, P + 1)
                if P >= 2:
                    a.wait_ge(s_ydma, 16 * (P - 1))
                ab = abase(P)
                a.activation(
                    tmps[P % 2][:],
                    acc48[ab:ab + 16, :, :].rearrange("p a b -> p (a b)"),
                    AF.Identity, bias=cOFF, scale=1.0,
                    accum_out=s1cols[:, P:P + 1]).then_inc(s_ev, 1)
                # self-sync: the square and the DMA read tmp the evac just
                # wrote; wait for the evac write to retire first
                a.wait_ge(s_ev, P + 1)
                a.activation(
                    ysq[:], tmps[P % 2][:], AF.Square,
                    accum_out=s2cols[:, P:P + 1]).then_inc(s_sq, 1)
                a.wait_ge(s_sq, P + 1)
                a.dma_start(ybuf[:, P, :],
                            tmps[P % 2][:]).then_inc(s_ydma, 16)

            for n in range(N):
                for h in range(2):
                    P = 2 * n + h
                    if h == 0:
                        a.wait_ge(s_dmaxs[n % 2], 16 * (n // 2 + 1))
                    for j in range(CP):
                        st = P * 8 + j
                        for k in range(3):  # taps (2,0),(2,1),(2,2)
                            aidx = st * 3 + k
                            if aidx - (RA_N - 1) > 0:
                                a.wait_ge(s_sg, aidx - (RA_N - 1))
                            a.activation(
                                RA[aidx % RA_N][:, :, 0:128],
                                src(n, h, 2)[:, :, k:k + 128], AF.Abs,
                                bias=wcol(j, 6 + k),
                                scale=-1.0).then_inc(s_Ta, 1)
                        if j == 1 and P >= 1:
                            evac(P - 1)
            evac(NPASS - 1)

            # ---- BN tail ----
            a.wait_ge(s_pe, 1)
            a.mul(mean8[:], acc48[0:8, 0, 0:1], 1.0 / CNT).then_inc(s_ac, 1)
            a.wait_ge(s_pe, 2)
            a.mul(ey28[:], acc48[0:8, 1, 0:1], 1.0 / CNT).then_inc(s_ac, 1)
            a.wait_ge(s_dv, 3)
            a.activation(sqt[:], var8[:], AF.Sqrt).then_inc(s_ac, 1)
            a.wait_ge(s_dv, 4)
            a.dma_start(bnscr[0:1, 0:16], scsh8[:]).then_inc(s_bn, 16)
            a.wait_ge(s_bn, 16)
            # AB16[2j+b] = (A_j, B_j)
            a.dma_start(
                AB16[:],
                bnscr[0:1, 0:16].rearrange("a (j e) -> a j e", e=2)
                .unsqueeze(2).broadcast_to([1, 8, 2, 2])).then_inc(s_bn, 16)
            # affine: load ybuf chunk -> DVE affine -> store to out
            # (interleaved: the in-order ACT queue must not emit a load
            # whose wait depends on a store emitted later)
            a.wait_ge(s_ydma, 16 * NPASS)

            def outdma(cch):
                nn, hh = cch // 2, cch % 2
                dst = out[8 * nn:8 * nn + 8, :].rearrange(
                    "p (b h2 r c) -> p b h2 (r c)", b=2, h2=2, r=32
                )[:, :, hh, :]
                a.dma_start(dst, tmps[cch % 2][:]).then_inc(s_bn, 16)

            for cch in range(NPASS):
                if cch >= 2:
                    a.wait_ge(s_bn, 32 + 16 * (cch - 1))
                a.dma_start(tmps[cch % 2][:],
                            ybuf[:, cch, :]).then_inc(s_yin, 16)
                if cch >= 1:
                    a.wait_ge(s_p3, cch)
                    outdma(cch - 1)
            a.wait_ge(s_p3, NPASS)
            outdma(NPASS - 1)
            a.wait_ge(s_bn, 32 + 16 * NPASS)

    return nc


_LAST_RESULTS = None


def _host_inputs(x, weight, gamma, beta):
    x = np.ascontiguousarray(np.asarray(x, dtype=np.float32))
    weight = np.asarray(weight, dtype=np.float32)
    gamma = np.asarray(gamma, dtype=np.float32)
    beta = np.asarray(beta, dtype=np.float32)

    x16 = x.astype(np.float16)
    x16p = np.zeros((N, 128, ROWS, RW), np.float16)
    x16p[:, 0:64, 1:66, 1:129] = x16[:, :, 0:65, :]
    x16p[:, 64:128, 0:65, 1:129] = x16[:, :, 63:128, :]
    x16p = x16p.reshape(N, 128, ROWS * RW)

    in_maps = []
    for core in range(NCORES):
        cs = slice(CP * core, CP * (core + 1))
        wslice = weight[cs]
        warr = np.tile(
            wslice.transpose(1, 0, 2, 3).reshape(64, CP * 9), (2, 1)
        ).astype(np.float32)
        c32 = np.zeros((128, NC32), np.float32)
        c32[:, 0:CP * 9] = warr
        c32[0:8, COL_G] = gamma[cs]
        c32[0:8, COL_B] = beta[cs]
        c32[np.arange(16), COL_S1F + np.arange(16) // 2] = 1.0
        for j in range(CP):
            wf = wslice[j].reshape(64, 9).astype(np.float64)
            e_abs = 0.0
            for ci in range(64):
                for t in range(9):
                    wv = float(wf[ci, t])
                    e_abs += (math.sqrt(2.0 / math.pi)
                              * math.exp(-0.5 * wv * wv)
                              + wv * math.erf(wv / math.sqrt(2.0)))
            # evac bias: E[sum|x-w|] + sum_{min taps} w centers fp16 range
            coff = e_abs + float(wf[:, 0:6].sum())
            c32[2 * j, COL_OFF] = coff
            c32[2 * j + 1, COL_OFF] = coff

        selmm = np.zeros((128, NSEL, 16), np.float16)
        for b in range(2):
            rows = slice(b * 64, (b + 1) * 64)
            for j in range(CP):
                selmm[rows, SEL_PAIR0 + j, 2 * j + b] = 2.0
                selmm[rows, SEL_ABS0 + j, 2 * j + b] = -1.0
            selmm[rows, SEL_BOX, b::2] = -1.0
        for j in range(CP):
            cog = CP * core + j
            for b in range(2):
                selmm[b * 64 + cog, SEL_RES, 2 * j + b] = 1.0
        in_maps.append({
            "x16p": x16p,
            "consts32": c32,
            "selmm": selmm,
        })
    return in_maps


def kernel(x, weight, gamma, beta, alpha):
    from concourse.bass_utils import run_bass_kernel_spmd

    nc = _build_program()
    in_maps = _host_inputs(x, weight, gamma, beta)

    trace = os.environ.get("ADDER_TRACE", "0") == "1"
    if os.environ.get("ADDER_WARMUP", "1") == "1":
        try:
            run_bass_kernel_spmd(nc, in_maps, core_ids=list(range(NCORES)),
                                 trace=False)
        except Exception:
            pass
    res = run_bass_kernel_spmd(nc, in_maps, core_ids=list(range(NCORES)),
                               trace=trace)
    global _LAST_RESULTS
    _LAST_RESULTS = res

    # out rows 8n+j; pixel order (b, h, r, c) row-major = plain (h, w)
    parts = [r["out"].astype(np.float32).reshape(N, CP, H, W)
             for r in res.results]
    full = np.concatenate(parts, axis=1).astype(np.float32)

    a = float(np.asarray(alpha))
    if a != 1.0:
        full = np.sign(full) * np.power(np.abs(full) + 1e-12, a,
                                        dtype=np.float32)
    return full


# revision 26
# speedup vs baseline: 1.1882x; 1.1882x over previous
"""AdderNet layer (adder2d + residual + BatchNorm(train) + PowerActivation)
on 8 Trainium2 NeuronCores. Raw Bass, explicit semaphores.

v4: all-abs production via the abs_max ALU trick (|x-w| = abs_max(x-w, 0)
in ONE DVE tensor_scalar at the 4x perf mode), giving every tap tile a
uniform PE coefficient of -1. This removes the v3 box-sum passes and the
min/abs algebra split. 32-row tiles (2x fewer elementwise ops than v3's
16-row groups -> amortized per-op overhead), PSUM laid out as
[16 outs, 8 banks] per 32-row pass (two pass buffers at partition bases
0/32 of one [48,8,512] tensor), so evac is ONE activation per pass, and
Sum(y^2) is accumulated during evac (kills the separate variance pass
and the mean64 broadcast). The fp16 y buffer lives in DRAM (frees
~30KB/partition of SBUF); the BN affine tail streams it back through
the freed tmp buffers.

Per (j, 32-row pass): 6 DVE abs tiles + 3 ACT abs tiles; DVE pre-merges
the vertical pairs (kh0,kw)+(kh1,kw) for kw=0,1 -> 7 PE clusters of 8
matmuls each + shared residual. Engine busy model per pass: DVE ~95us,
ACT ~97us, PE ~97us - balanced within ~3%.

Self-contained: hardcodes N,C,H,W=8,64,128,128, CO=64, K=3, pad=1.
Sharding by OUTPUT CHANNEL (8 co per core): BN stats core-local.
PowerActivation alpha=1.0 is identity (harness value); exact host
fallback otherwise. kernel() runs a warmup execution first.
"""

import math
import os
from contextlib import ExitStack

import numpy as np

N, C, H, W = 8, 64, 128, 128
CO, KS = 64, 3
BN_EPS = 1e-5
NCORES = 8
CP = CO // NCORES
RW = 132
ROWS = 66
PIX = H * W
CNT = float(N * PIX)
NPASS = 2 * N            # 16 32-row passes

# ring sizes (tiles of [128, 32, <=130] fp16, ~8.3 KB/partition)
RD_N = 4    # DVE pair-feed tiles (consumed by DVE merges, same queue)
RP_N = 4    # merged pair tiles -> PE
RS_N = 4    # DVE single tiles (kw=2) -> PE
RA_N = 4    # ACT abs tiles -> PE

SEL_PAIR0 = 0             # selmm slices: 0..7 pair/min lhs (+2) per j
SEL_ABS0 = 8              # 8..15 abs lhs (-1) per j
SEL_BOX = 16              # -1 box-sum, all j columns
SEL_RES = 17              # +1 residual
NSEL = 18

COL_G = 72
COL_B = 73
COL_OFF = 74
COL_S1F = 76              # sel: [2j+b, j] = 1 (rows 0:16, 8 cols)
NC32 = 92

TS_PER_PASS = 6 * CP      # DVE tensor_scalar productions per 32-row pass
TA_PER_PASS = 3 * CP      # ACT productions per pass


def _build_program():
    import concourse.bass as bass
    import concourse.mybir as mybir
    from concourse.mybir import AluOpType as Op

    f32 = mybir.dt.float32
    f16 = mybir.dt.float16
    AF = mybir.ActivationFunctionType

    nc = bass.Bass("TRN2")

    x16p = nc.dram_tensor("x16p", [N, 128, ROWS * RW], f16,
                          kind="ExternalInput")
    consts32 = nc.dram_tensor("consts32", [128, NC32], f32,
                              kind="ExternalInput")
    selmm = nc.dram_tensor("selmm", [128, NSEL, 16], f16,
                           kind="ExternalInput")
    out = nc.dram_tensor("out", [64, PIX], f16, kind="ExternalOutput")
    ybuf = nc.dram_tensor("ybuf", [16, NPASS, 4096], f16, kind="Internal")
    bnscr = nc.dram_tensor("bnscr", [1, 16], f32, kind="Internal")

    ctx = ExitStack()
    with ctx:
        c32 = ctx.enter_context(nc.sbuf_tensor("c32", [128, NC32], f32))
        selmm_sb = ctx.enter_context(
            nc.sbuf_tensor("selmm_sb", [128, NSEL, 16], f16))
        xpad0 = ctx.enter_context(nc.sbuf_tensor("xpad0", [128, ROWS, RW], f16))
        xpad1 = ctx.enter_context(nc.sbuf_tensor("xpad1", [128, ROWS, RW], f16))
        xpads = [xpad0, xpad1]
        RD = [ctx.enter_context(nc.sbuf_tensor(f"RD{i}", [128, 32, 132], f16))
              for i in range(RD_N)]
        RP = [ctx.enter_context(nc.sbuf_tensor(f"RP{i}", [128, 32, 132], f16))
              for i in range(RP_N)]
        RS = [ctx.enter_context(nc.sbuf_tensor(f"RS{i}", [128, 32, 132], f16))
              for i in range(RS_N)]
        RA = [ctx.enter_context(nc.sbuf_tensor(f"RA{i}", [128, 32, 132], f16))
              for i in range(RA_N)]
        xvs = [ctx.enter_context(nc.sbuf_tensor(f"xv{i}", [128, 32, 132], f16))
               for i in range(2)]
        tmp0 = ctx.enter_context(nc.sbuf_tensor("tmp0", [16, 4096], f16))
        tmp1 = ctx.enter_context(nc.sbuf_tensor("tmp1", [16, 4096], f16))
        tmps = [tmp0, tmp1]

        s1cols = ctx.enter_context(nc.sbuf_tensor("s1cols", [16, NPASS], f32))
        s2cols = ctx.enter_context(nc.sbuf_tensor("s2cols", [16, NPASS], f32))
        s1p = ctx.enter_context(nc.sbuf_tensor("s1p", [16, 1], f32))
        s2p = ctx.enter_context(nc.sbuf_tensor("s2p", [16, 1], f32))
        mean8 = ctx.enter_context(nc.sbuf_tensor("mean8", [8, 1], f32))
        ey28 = ctx.enter_context(nc.sbuf_tensor("ey28", [8, 1], f32))
        msq = ctx.enter_context(nc.sbuf_tensor("msq", [8, 1], f32))
        var8 = ctx.enter_context(nc.sbuf_tensor("var8", [8, 1], f32))
        sqt = ctx.enter_context(nc.sbuf_tensor("sqt", [8, 1], f32))
        rt = ctx.enter_context(nc.sbuf_tensor("rt", [8, 1], f32))
        ut = ctx.enter_context(nc.sbuf_tensor("ut", [8, 1], f32))
        scsh8 = ctx.enter_context(nc.sbuf_tensor("scsh8", [8, 2], f32))
        AB16 = ctx.enter_context(nc.sbuf_tensor("AB16", [16, 2], f32))

        acc48 = ctx.enter_context(nc.psum_tensor("acc48", [48, 8, 512], f32))

        s_dmac = ctx.enter_context(nc.semaphore())
        s_dmax0 = ctx.enter_context(nc.semaphore())
        s_dmax1 = ctx.enter_context(nc.semaphore())
        s_dmaxs = [s_dmax0, s_dmax1]
        s_Td = ctx.enter_context(nc.semaphore())   # DVE TS productions
        s_Ta = ctx.enter_context(nc.semaphore())   # ACT productions
        s_pr = ctx.enter_context(nc.semaphore())   # DVE pair merges
        s_pg = ctx.enter_context(nc.semaphore())   # PE consumed pair
        s_ms = ctx.enter_context(nc.semaphore())   # PE consumed RS single
        s_sg = ctx.enter_context(nc.semaphore())   # PE consumed ACT tile
        s_ev = ctx.enter_context(nc.semaphore())   # evac done (per pass)
        s_sq = ctx.enter_context(nc.semaphore())   # square done (per pass)
        s_ev2 = ctx.enter_context(nc.semaphore())  # PE pass drain
        s_ydma = ctx.enter_context(nc.semaphore())  # ybuf store done
        s_pe = ctx.enter_context(nc.semaphore())   # PE BN folds
        s_dv = ctx.enter_context(nc.semaphore())   # DVE BN steps
        s_ac = ctx.enter_context(nc.semaphore())   # ACT BN steps
        s_vc = ctx.enter_context(nc.semaphore())   # DVE serial chain
        s_bn = ctx.enter_context(nc.semaphore())   # BN dma chain
        s_yin = ctx.enter_context(nc.semaphore())  # affine chunk loads
        s_p3 = ctx.enter_context(nc.semaphore())   # affine chunks done
        s_xv = ctx.enter_context(nc.semaphore())   # xv tile produced
        s_xvc = ctx.enter_context(nc.semaphore())  # PE consumed xv
        block = ctx.enter_context(nc.Block())

        gma = c32[0:8, COL_G:COL_G + 1]
        bta = c32[0:8, COL_B:COL_B + 1]
        cOFF = c32[0:16, COL_OFF:COL_OFF + 1]

        def wcol(j, t):
            return c32[:, j * 9 + t:j * 9 + t + 1]

        def src(n, h, kh):
            base = 32 * h + kh
            return xpads[n % 2][:, base:base + 32, :]

        def abase(P):
            return 32 * (P % 2)

        # global stage index: stage = P * 8 + j, P = 2n + h
        # ---------------- Pool: DMA loads ----------------
        @block.gpsimd
        def _(gp):
            gp.dma_start(c32[:], consts32[:]).then_inc(s_dmac, 16)
            gp.dma_start(selmm_sb[:], selmm[:]).then_inc(s_dmac, 16)
            for n in range(2):
                gp.dma_start(
                    xpads[n][:].rearrange("p r c -> p (r c)"),
                    x16p[n, :, :]).then_inc(s_dmaxs[n], 16)
            for nl in range(2, N):
                # image nl-2's readers must be done before overwrite
                gp.wait_ge(s_ev2, 2 * (nl - 1))
                gp.wait_ge(s_Td, TS_PER_PASS * 2 * (nl - 1))
                gp.wait_ge(s_Ta, TA_PER_PASS * 2 * (nl - 1))
                gp.dma_start(
                    xpads[nl % 2][:].rearrange("p r c -> p (r c)"),
                    x16p[nl, :, :]).then_inc(s_dmaxs[nl % 2], 16)

        # ---------------- DVE: abs tiles + pair merges + BN ----------
        @block.vector
        def _(v_):
            v_.wait_ge(s_dmac, 32)
            for n in range(N):
                for h in range(2):
                    P = 2 * n + h
                    if h == 0:
                        v_.wait_ge(s_dmaxs[n % 2], 16 * (n // 2 + 1))
                    # xv: row-pair sum for the box passes
                    if P >= 2:
                        v_.wait_ge(s_xvc, 3 * (P - 1))
                    v_.tensor_tensor(
                        xvs[P % 2][:, :, 0:131],
                        src(n, h, 0)[:, :, 0:131],
                        src(n, h, 1)[:, :, 0:131],
                        Op.add).then_inc(s_xv, 1)
                    for j in range(CP):
                        st = P * 8 + j
                        # pair feeds: taps (0,0),(0,1),(1,0),(1,1)
                        for mi, (kh, kw) in enumerate(
                                ((0, 0), (0, 1), (1, 0), (1, 1))):
                            fidx = st * 4 + mi
                            v_.tensor_scalar(
                                RD[fidx % RD_N][:, :, 0:130],
                                src(n, h, kh)[:, :, 0:130],
                                wcol(j, 3 * kh + kw), 0.0,
                                Op.subtract, Op.min).then_inc(s_Td, 1)
                        # singles: taps (0,2),(1,2) — emitted between the
                        # feeds and the merges so the merge reads are >2us
                        # behind the feed writes (same-engine RAW pipeline)
                        for i in range(2):
                            sidx = st * 2 + i
                            if sidx - (RS_N - 1) > 0:
                                v_.wait_ge(s_ms, sidx - (RS_N - 1))
                            v_.tensor_scalar(
                                RS[sidx % RS_N][:, :, 2:130],
                                src(n, h, i)[:, :, 2:130],
                                wcol(j, 3 * i + 2), 0.0,
                                Op.subtract, Op.min).then_inc(s_Td, 1)
                        for k in range(2):  # merge (0,k)+(1,k)
                            pidx = st * 2 + k
                            if pidx - (RP_N - 1) > 0:
                                v_.wait_ge(s_pg, pidx - (RP_N - 1))
                            fa = st * 4 + k
                            fb = st * 4 + 2 + k
                            v_.tensor_tensor(
                                RP[pidx % RP_N][:, :, 0:130],
                                RD[fa % RD_N][:, :, 0:130],
                                RD[fb % RD_N][:, :, 0:130],
                                Op.add).then_inc(s_pr, 1)

            # ---- BN tail ----
            v_.wait_ge(s_ev, NPASS)
            v_.tensor_reduce(s1p[:], s1cols[:], mybir.AxisListType.X,
                             Op.add).then_inc(s_dv, 1)
            v_.wait_ge(s_sq, NPASS)
            v_.tensor_reduce(s2p[:], s2cols[:], mybir.AxisListType.X,
                             Op.add).then_inc(s_dv, 1)
            v_.wait_ge(s_ac, 2)
            vcnt = 0

            def vstep(inst):
                nonlocal vcnt
                vcnt += 1
                inst.then_inc(s_vc, 1)
                v_.wait_ge(s_vc, vcnt)

            vstep(v_.tensor_tensor(msq[:], mean8[:], mean8[:], Op.mult))
            vstep(v_.tensor_tensor(var8[:], ey28[:], msq[:], Op.subtract))
            v_.tensor_scalar_add(var8[:], var8[:], BN_EPS).then_inc(s_dv, 1)
            v_.wait_ge(s_ac, 3)
            vstep(v_.reciprocal(rt[:], sqt[:]))
            for _i in range(2):
                vstep(v_.tensor_tensor(ut[:], rt[:], rt[:], Op.mult))
                vstep(v_.tensor_tensor(ut[:], ut[:], var8[:], Op.mult))
                vstep(v_.tensor_scalar(ut[:], ut[:], -0.5, 1.5,
                                       Op.mult, Op.add))
                vstep(v_.tensor_tensor(rt[:], rt[:], ut[:], Op.mult))
            vstep(v_.tensor_tensor(scsh8[:, 0:1], gma, rt[:], Op.mult))
            vstep(v_.tensor_tensor(scsh8[:, 1:2], mean8[:], scsh8[:, 0:1],
                                   Op.mult))
            v_.tensor_tensor(scsh8[:, 1:2], bta, scsh8[:, 1:2],
                             Op.subtract).then_inc(s_dv, 1)
            # affine chunks (one pass each) in the freed tmp buffers
            v_.wait_ge(s_bn, 32)
            for cch in range(NPASS):
                v_.wait_ge(s_yin, 16 * (cch + 1))
                v_.tensor_scalar(
                    tmps[cch % 2][:], tmps[cch % 2][:],
                    AB16[:, 0:1], AB16[:, 1:2],
                    Op.mult, Op.add).then_inc(s_p3, 1)

        # ---------------- PE: reduction matmuls ----------------
        @block.tensor
        def _(t_):
            t_.wait_ge(s_dmac, 32)
            for n in range(N):
                for h in range(2):
                    P = 2 * n + h
                    ab = abase(P)
                    if P >= 2:
                        t_.wait_ge(s_ev, P - 1)
                    if h == 0:
                        t_.wait_ge(s_dmaxs[n % 2], 16 * (n // 2 + 1))
                    # residual: start=True zeroes this pass's banks
                    for rq in range(2):
                        for cb in range(4):
                            t_.matmul(
                                acc48[ab:ab + 16, 4 * rq + cb, :],
                                selmm_sb[:, SEL_RES, :],
                                xpads[n % 2][:, 1 + 32 * h + 16 * rq + 4 * cb:
                                             1 + 32 * h + 16 * rq + 4 * cb + 4,
                                             1:129],
                                start=True, stop=False,
                                skip_group_check=True)

                    def unit(tile, sel, colsl, sem, is_last):
                        for rq in range(2):
                            for cb in range(4):
                                mm = t_.matmul(
                                    acc48[ab:ab + 16, 4 * rq + cb, :],
                                    selmm_sb[:, sel, :],
                                    tile[:, 16 * rq + 4 * cb:
                                         16 * rq + 4 * cb + 4, colsl],
                                    start=False,
                                    stop=is_last and rq == 1 and cb == 3,
                                    skip_group_check=True)
                                if rq == 1 and cb == 3:
                                    mm.then_inc(sem, 1)

                    # box passes: -x over the 6 min taps via xv
                    t_.wait_ge(s_xv, P + 1)
                    unit(xvs[P % 2], SEL_BOX, slice(0, 128), s_xvc, False)
                    unit(xvs[P % 2], SEL_BOX, slice(1, 129), s_xvc, False)
                    unit(xvs[P % 2], SEL_BOX, slice(2, 130), s_xvc, False)
                    for j in range(CP):
                        st = P * 8 + j
                        last_j = (j == CP - 1)
                        for k in range(2):  # pairs
                            pidx = st * 2 + k
                            t_.wait_ge(s_pr, pidx + 1)
                            unit(RP[pidx % RP_N], SEL_PAIR0 + j,
                                 slice(k, k + 128), s_pg, False)
                        for i in range(2):  # DVE singles
                            t_.wait_ge(s_Td, st * 6 + 5 + i)
                            sidx = st * 2 + i
                            unit(RS[sidx % RS_N], SEL_PAIR0 + j,
                                 slice(2, 130), s_ms, False)
                        for k in range(3):  # ACT tiles
                            aidx = st * 3 + k
                            t_.wait_ge(s_Ta, aidx + 1)
                            unit(RA[aidx % RA_N], SEL_ABS0 + j,
                                 slice(k, k + 128), s_sg, last_j and k == 2)
                    t_.drain().then_inc(s_ev2, 1)
            # BN folds: [16] -> [8] selection matmuls
            t_.wait_ge(s_dv, 1)
            t_.matmul(acc48[0:8, 0, 0:1], c32[0:16, COL_S1F:COL_S1F + 8],
                      s1p[:], start=True, stop=True,
                      skip_group_check=True).then_inc(s_pe, 1)
            t_.wait_ge(s_dv, 2)
            t_.matmul(acc48[0:8, 1, 0:1], c32[0:16, COL_S1F:COL_S1F + 8],
                      s2p[:], start=True, stop=True,
                      skip_group_check=True).then_inc(s_pe, 1)

        # ---------------- ACT: abs tiles + evac + BN tail -------------
        @block.scalar
        def _(a):
            a.wait_ge(s_dmac, 32)

            def evac(P):
                # pass P: wait PE drain, evacuate acc -> tmp, square, store
                a.wait_ge(s_ev2, P + 1)
                if P >= 2:
                    a.wait_ge(s_ydma, 16 * (P - 1))
                ab = abase(P)
                a.activation(
                    tmps[P % 2][:],
                    acc48[ab:ab + 16, :, :].rearrange("p a b -> p (a b)"),
                    AF.Identity, bias=cOFF, scale=1.0,
                    accum_out=s1cols[:, P:P + 1]).then_inc(s_ev, 1)
                # self-sync: the square and the DMA read tmp the evac just
                # wrote; wait for the evac write to retire first
                a.wait_ge(s_ev, P + 1)
                # square scratch = the other tmp (its ybuf store is done)
                if P >= 1:
                    a.wait_ge(s_ydma, 16 * P)
                a.activation(
                    tmps[(P + 1) % 2][:], tmps[P % 2][:], AF.Square,
                    accum_out=s2cols[:, P:P + 1]).then_inc(s_sq, 1)
                a.wait_ge(s_sq, P + 1)
                a.dma_start(ybuf[:, P, :],
                            tmps[P % 2][:]).then_inc(s_ydma, 16)

            for n in range(N):
                for h in range(2):
                    P = 2 * n + h
                    if h == 0:
                        a.wait_ge(s_dmaxs[n % 2], 16 * (n // 2 + 1))
                    for j in range(CP):
                        st = P * 8 + j
                        for k in range(3):  # taps (2,0),(2,1),(2,2)
                            aidx = st * 3 + k
                            if aidx - (RA_N - 1) > 0:
                                a.wait_ge(s_sg, aidx - (RA_N - 1))
                            a.activation(
                                RA[aidx % RA_N][:, :, 0:131],
                                src(n, h, 2)[:, :, 0:131], AF.Abs,
                                bias=wcol(j, 6 + k),
                                scale=-1.0).then_inc(s_Ta, 1)
                        if j == 1 and P >= 1:
                            evac(P - 1)
            evac(NPASS - 1)

            # ---- BN tail ----
            a.wait_ge(s_pe, 1)
            a.mul(mean8[:], acc48[0:8, 0, 0:1], 1.0 / CNT).then_inc(s_ac, 1)
            a.wait_ge(s_pe, 2)
            a.mul(ey28[:], acc48[0:8, 1, 0:1], 1.0 / CNT).then_inc(s_ac, 1)
            a.wait_ge(s_dv, 3)
            a.activation(sqt[:], var8[:], AF.Sqrt).then_inc(s_ac, 1)
            a.wait_ge(s_dv, 4)
            a.dma_start(bnscr[0:1, 0:16], scsh8[:]).then_inc(s_bn, 16)
            a.wait_ge(s_bn, 16)
            # AB16[2j+b] = (A_j, B_j)
            a.dma_start(
                AB16[:],
                bnscr[0:1, 0:16].rearrange("a (j e) -> a j e", e=2)
                .unsqueeze(2).broadcast_to([1, 8, 2, 2])).then_inc(s_bn, 16)
            # affine: load ybuf chunk -> DVE affine -> store to out
            # (interleaved: the in-order ACT queue must not emit a load
            # whose wait depends on a store emitted later)
            a.wait_ge(s_ydma, 16 * NPASS)

            def outdma(cch):
                nn, hh = cch // 2, cch % 2
                dst = out[8 * nn:8 * nn + 8, :].rearrange(
                    "p (b h2 r c) -> p b h2 (r c)", b=2, h2=2, r=32
                )[:, :, hh, :]
                a.dma_start(dst, tmps[cch % 2][:]).then_inc(s_bn, 16)

            for cch in range(NPASS):
                if cch >= 2:
                    a.wait_ge(s_bn, 32 + 16 * (cch - 1))
                a.dma_start(tmps[cch % 2][:],
                            ybuf[:, cch, :]).then_inc(s_yin, 16)
                if cch >= 1:
                    a.wait_ge(s_p3, cch)
                    outdma(cch - 1)
            a.wait_ge(s_p3, NPASS)
            outdma(NPASS - 1)
            a.wait_ge(s_bn, 32 + 16 * NPASS)

    return nc


_LAST_RESULTS = None


def _host_inputs(x, weight, gamma, beta):
    x = np.ascontiguousarray(np.asarray(x, dtype=np.float32))
    weight = np.asarray(weight, dtype=np.float32)
    gamma = np.asarray(gamma, dtype=np.float32)
    beta = np.asarray(beta, dtype=np.float32)

    x16 = x.astype(np.float16)
    x16p = np.zeros((N, 128, ROWS, RW), np.float16)
    x16p[:, 0:64, 1:66, 1:129] = x16[:, :, 0:65, :]
    x16p[:, 64:128, 0:65, 1:129] = x16[:, :, 63:128, :]
    x16p = x16p.reshape(N, 128, ROWS * RW)

    in_maps = []
    for core in range(NCORES):
        cs = slice(CP * core, CP * (core + 1))
        wslice = weight[cs]
        warr = np.tile(
            wslice.transpose(1, 0, 2, 3).reshape(64, CP * 9), (2, 1)
        ).astype(np.float32)
        c32 = np.zeros((128, NC32), np.float32)
        c32[:, 0:CP * 9] = warr
        c32[0:8, COL_G] = gamma[cs]
        c32[0:8, COL_B] = beta[cs]
        c32[np.arange(16), COL_S1F + np.arange(16) // 2] = 1.0
        for j in range(CP):
            wf = wslice[j].reshape(64, 9).astype(np.float64)
            e_abs = 0.0
            for ci in range(64):
                for t in range(9):
                    wv = float(wf[ci, t])
                    e_abs += (math.sqrt(2.0 / math.pi)
                              * math.exp(-0.5 * wv * wv)
                              + wv * math.erf(wv / math.sqrt(2.0)))
            # evac bias: E[sum|x-w|] + sum_{min taps} w centers fp16 range
            coff = e_abs + float(wf[:, 0:6].sum())
            c32[2 * j, COL_OFF] = coff
            c32[2 * j + 1, COL_OFF] = coff

        selmm = np.zeros((128, NSEL, 16), np.float16)
        for b in range(2):
            rows = slice(b * 64, (b + 1) * 64)
            for j in range(CP):
                selmm[rows, SEL_PAIR0 + j, 2 * j + b] = 2.0
                selmm[rows, SEL_ABS0 + j, 2 * j + b] = -1.0
            selmm[rows, SEL_BOX, b::2] = -1.0
        for j in range(CP):
            cog = CP * core + j
            for b in range(2):
                selmm[b * 64 + cog, SEL_RES, 2 * j + b] = 1.0
        in_maps.append({
            "x16p": x16p,
            "consts32": c32,
            "selmm": selmm,
        })
    return in_maps


def kernel(x, weight, gamma, beta, alpha):
    from concourse.bass_utils import run_bass_kernel_spmd

    nc = _build_program()
    in_maps = _host_inputs(x, weight, gamma, beta)

    trace = os.environ.get("ADDER_TRACE", "0") == "1"
    if os.environ.get("ADDER_WARMUP", "1") == "1":
        try:
            run_bass_kernel_spmd(nc, in_maps, core_ids=list(range(NCORES)),
                                 trace=False)
        except Exception:
            pass
    res = run_bass_kernel_spmd(nc, in_maps, core_ids=list(range(NCORES)),
                               trace=trace)
    global _LAST_RESULTS
    _LAST_RESULTS = res

    # out rows 8n+j; pixel order (b, h, r, c) row-major = plain (h, w)
    parts = [r["out"].astype(np.float32).reshape(N, CP, H, W)
             for r in res.results]
    full = np.concatenate(parts, axis=1).astype(np.float32)

    a = float(np.asarray(alpha))
    if a != 1.0:
        full = np.sign(full) * np.power(np.abs(full) + 1e-12, a,
                                        dtype=np.float32)
    return full


# revision 32
# speedup vs baseline: 1.2029x; 1.0124x over previous
"""AdderNet layer (adder2d + residual + BatchNorm(train) + PowerActivation)
on 8 Trainium2 NeuronCores. Raw Bass, explicit semaphores.

v4: all-abs production via the abs_max ALU trick (|x-w| = abs_max(x-w, 0)
in ONE DVE tensor_scalar at the 4x perf mode), giving every tap tile a
uniform PE coefficient of -1. This removes the v3 box-sum passes and the
min/abs algebra split. 32-row tiles (2x fewer elementwise ops than v3's
16-row groups -> amortized per-op overhead), PSUM laid out as
[16 outs, 8 banks] per 32-row pass (two pass buffers at partition bases
0/32 of one [48,8,512] tensor), so evac is ONE activation per pass, and
Sum(y^2) is accumulated during evac (kills the separate variance pass
and the mean64 broadcast). The fp16 y buffer lives in DRAM (frees
~30KB/partition of SBUF); the BN affine tail streams it back through
the freed tmp buffers.

Per (j, 32-row pass): 6 DVE abs tiles + 3 ACT abs tiles; DVE pre-merges
the vertical pairs (kh0,kw)+(kh1,kw) for kw=0,1 -> 7 PE clusters of 8
matmuls each + shared residual. Engine busy model per pass: DVE ~95us,
ACT ~97us, PE ~97us - balanced within ~3%.

Self-contained: hardcodes N,C,H,W=8,64,128,128, CO=64, K=3, pad=1.
Sharding by OUTPUT CHANNEL (8 co per core): BN stats core-local.
PowerActivation alpha=1.0 is identity (harness value); exact host
fallback otherwise. kernel() runs a warmup execution first.
"""

import math
import os
from contextlib import ExitStack

import numpy as np

N, C, H, W = 8, 64, 128, 128
CO, KS = 64, 3
BN_EPS = 1e-5
NCORES = 8
CP = CO // NCORES
RW = 132
ROWS = 66
PIX = H * W
CNT = float(N * PIX)
NPASS = 2 * N            # 16 32-row passes

# ring sizes (tiles of [128, 32, <=130] fp16, ~8.3 KB/partition)
RD_N = 4    # DVE pair-feed tiles (consumed by DVE merges, same queue)
RP_N = 4    # merged pair tiles -> PE
RS_N = 4    # DVE single tiles (kw=2) -> PE
RA_N = 4    # ACT abs tiles -> PE

SEL_PAIR0 = 0             # selmm slices: 0..7 pair/min lhs (+2) per j
SEL_ABS0 = 8              # 8..15 abs lhs (-1) per j
SEL_BOX = 16              # -1 box-sum, all j columns
SEL_RES = 17              # +1 residual
NSEL = 18

COL_G = 72
COL_B = 73
COL_OFF = 74
COL_S1F = 76              # sel: [2j+b, j] = 1 (rows 0:16, 8 cols)
NC32 = 92

TS_PER_PASS = 6 * CP      # DVE tensor_scalar productions per 32-row pass
TA_PER_PASS = 3 * CP      # ACT productions per pass


def _build_program():
    import concourse.bass as bass
    import concourse.mybir as mybir
    from concourse.mybir import AluOpType as Op

    f32 = mybir.dt.float32
    f16 = mybir.dt.float16
    AF = mybir.ActivationFunctionType

    nc = bass.Bass("TRN2")

    x16p = nc.dram_tensor("x16p", [N, 128, ROWS * RW], f16,
                          kind="ExternalInput")
    consts32 = nc.dram_tensor("consts32", [128, NC32], f32,
                              kind="ExternalInput")
    selmm = nc.dram_tensor("selmm", [128, NSEL, 16], f16,
                           kind="ExternalInput")
    out = nc.dram_tensor("out", [64, PIX], f16, kind="ExternalOutput")
    ybuf = nc.dram_tensor("ybuf", [16, NPASS, 4096], f16, kind="Internal")
    bnscr = nc.dram_tensor("bnscr", [1, 16], f32, kind="Internal")

    ctx = ExitStack()
    with ctx:
        c32 = ctx.enter_context(nc.sbuf_tensor("c32", [128, NC32], f32))
        selmm_sb = ctx.enter_context(
            nc.sbuf_tensor("selmm_sb", [128, NSEL, 16], f16))
        xpad0 = ctx.enter_context(nc.sbuf_tensor("xpad0", [128, ROWS, RW], f16))
        xpad1 = ctx.enter_context(nc.sbuf_tensor("xpad1", [128, ROWS, RW], f16))
        xpads = [xpad0, xpad1]
        RD = [ctx.enter_context(nc.sbuf_tensor(f"RD{i}", [128, 32, 132], f16))
              for i in range(RD_N)]
        RP = [ctx.enter_context(nc.sbuf_tensor(f"RP{i}", [128, 32, 132], f16))
              for i in range(RP_N)]
        RS = [ctx.enter_context(nc.sbuf_tensor(f"RS{i}", [128, 32, 132], f16))
              for i in range(RS_N)]
        RA = [ctx.enter_context(nc.sbuf_tensor(f"RA{i}", [128, 32, 132], f16))
              for i in range(RA_N)]
        xvs = [ctx.enter_context(nc.sbuf_tensor(f"xv{i}", [128, 32, 132], f16))
               for i in range(2)]
        tmp0 = ctx.enter_context(nc.sbuf_tensor("tmp0", [16, 4096], f16))
        tmp1 = ctx.enter_context(nc.sbuf_tensor("tmp1", [16, 4096], f16))
        tmps = [tmp0, tmp1]

        s1cols = ctx.enter_context(nc.sbuf_tensor("s1cols", [16, NPASS], f32))
        s2cols = ctx.enter_context(nc.sbuf_tensor("s2cols", [16, NPASS], f32))
        s1p = ctx.enter_context(nc.sbuf_tensor("s1p", [16, 1], f32))
        s2p = ctx.enter_context(nc.sbuf_tensor("s2p", [16, 1], f32))
        mean8 = ctx.enter_context(nc.sbuf_tensor("mean8", [8, 1], f32))
        ey28 = ctx.enter_context(nc.sbuf_tensor("ey28", [8, 1], f32))
        msq = ctx.enter_context(nc.sbuf_tensor("msq", [8, 1], f32))
        var8 = ctx.enter_context(nc.sbuf_tensor("var8", [8, 1], f32))
        sqt = ctx.enter_context(nc.sbuf_tensor("sqt", [8, 1], f32))
        rt = ctx.enter_context(nc.sbuf_tensor("rt", [8, 1], f32))
        ut = ctx.enter_context(nc.sbuf_tensor("ut", [8, 1], f32))
        scsh8 = ctx.enter_context(nc.sbuf_tensor("scsh8", [8, 2], f32))
        AB16 = ctx.enter_context(nc.sbuf_tensor("AB16", [16, 2], f32))

        acc48 = ctx.enter_context(nc.psum_tensor("acc48", [48, 8, 512], f32))

        s_dmac = ctx.enter_context(nc.semaphore())
        s_dmax0 = ctx.enter_context(nc.semaphore())
        s_dmax1 = ctx.enter_context(nc.semaphore())
        s_dmaxs = [s_dmax0, s_dmax1]
        s_Td = ctx.enter_context(nc.semaphore())   # DVE TS productions
        s_Ta = ctx.enter_context(nc.semaphore())   # ACT productions
        s_pr = ctx.enter_context(nc.semaphore())   # DVE pair merges
        s_pg = ctx.enter_context(nc.semaphore())   # PE consumed pair
        s_ms = ctx.enter_context(nc.semaphore())   # PE consumed RS single
        s_sg = ctx.enter_context(nc.semaphore())   # PE consumed ACT tile
        s_ev = ctx.enter_context(nc.semaphore())   # evac done (per pass)
        s_sq = ctx.enter_context(nc.semaphore())   # square done (per pass)
        s_ev2 = ctx.enter_context(nc.semaphore())  # PE pass drain
        s_ydma = ctx.enter_context(nc.semaphore())  # ybuf store done
        s_pe = ctx.enter_context(nc.semaphore())   # PE BN folds
        s_dv = ctx.enter_context(nc.semaphore())   # DVE BN steps
        s_ac = ctx.enter_context(nc.semaphore())   # ACT BN steps
        s_vc = ctx.enter_context(nc.semaphore())   # DVE serial chain
        s_bn = ctx.enter_context(nc.semaphore())   # BN dma chain
        s_yin = ctx.enter_context(nc.semaphore())  # affine chunk loads
        s_p3 = ctx.enter_context(nc.semaphore())   # affine chunks done
        s_xv = ctx.enter_context(nc.semaphore())   # xv tile produced
        s_xvc = ctx.enter_context(nc.semaphore())  # PE consumed xv
        block = ctx.enter_context(nc.Block())

        gma = c32[0:8, COL_G:COL_G + 1]
        bta = c32[0:8, COL_B:COL_B + 1]
        cOFF = c32[0:16, COL_OFF:COL_OFF + 1]

        def wcol(j, t):
            return c32[:, j * 9 + t:j * 9 + t + 1]

        def src(n, h, kh):
            base = 32 * h + kh
            return xpads[n % 2][:, base:base + 32, :]

        def abase(P):
            return 32 * (P % 2)

        # global stage index: stage = P * 8 + j, P = 2n + h
        # ---------------- Pool: DMA loads ----------------
        @block.gpsimd
        def _(gp):
            gp.dma_start(c32[:], consts32[:]).then_inc(s_dmac, 16)
            gp.dma_start(selmm_sb[:], selmm[:]).then_inc(s_dmac, 16)
            for n in range(2):
                gp.dma_start(
                    xpads[n][:].rearrange("p r c -> p (r c)"),
                    x16p[n, :, :]).then_inc(s_dmaxs[n], 16)
            for nl in range(2, N):
                # image nl-2's readers must be done before overwrite
                gp.wait_ge(s_sg, 24 * 2 * (nl - 1))
                gp.wait_ge(s_Td, TS_PER_PASS * 2 * (nl - 1))
                gp.wait_ge(s_Ta, TA_PER_PASS * 2 * (nl - 1))
                gp.dma_start(
                    xpads[nl % 2][:].rearrange("p r c -> p (r c)"),
                    x16p[nl, :, :]).then_inc(s_dmaxs[nl % 2], 16)

        # ---------------- DVE: abs tiles + pair merges + BN ----------
        @block.vector
        def _(v_):
            v_.wait_ge(s_dmac, 32)
            for n in range(N):
                for h in range(2):
                    P = 2 * n + h
                    if h == 0:
                        v_.wait_ge(s_dmaxs[n % 2], 16 * (n // 2 + 1))
                    # xv: row-pair sum for the box passes
                    if P >= 2:
                        v_.wait_ge(s_xvc, 3 * (P - 1))
                    v_.tensor_tensor(
                        xvs[P % 2][:, :, 0:131],
                        src(n, h, 0)[:, :, 0:131],
                        src(n, h, 1)[:, :, 0:131],
                        Op.add).then_inc(s_xv, 1)
                    for j in range(CP):
                        st = P * 8 + j
                        # pair feeds: taps (0,0),(0,1),(1,0),(1,1)
                        for mi, (kh, kw) in enumerate(
                                ((0, 0), (0, 1), (1, 0), (1, 1))):
                            fidx = st * 4 + mi
                            v_.tensor_scalar(
                                RD[fidx % RD_N][:, :, 0:130],
                                src(n, h, kh)[:, :, 0:130],
                                wcol(j, 3 * kh + kw), 0.0,
                                Op.subtract, Op.min).then_inc(s_Td, 1)
                        # singles: taps (0,2),(1,2) — emitted between the
                        # feeds and the merges so the merge reads are >2us
                        # behind the feed writes (same-engine RAW pipeline)
                        for i in range(2):
                            sidx = st * 2 + i
                            if sidx - (RS_N - 1) > 0:
                                v_.wait_ge(s_ms, sidx - (RS_N - 1))
                            v_.tensor_scalar(
                                RS[sidx % RS_N][:, :, 2:130],
                                src(n, h, i)[:, :, 2:130],
                                wcol(j, 3 * i + 2), 0.0,
                                Op.subtract, Op.min).then_inc(s_Td, 1)
                        for k in range(2):  # merge (0,k)+(1,k)
                            pidx = st * 2 + k
                            if pidx - (RP_N - 1) > 0:
                                v_.wait_ge(s_pg, pidx - (RP_N - 1))
                            fa = st * 4 + k
                            fb = st * 4 + 2 + k
                            v_.tensor_tensor(
                                RP[pidx % RP_N][:, :, 0:130],
                                RD[fa % RD_N][:, :, 0:130],
                                RD[fb % RD_N][:, :, 0:130],
                                Op.add).then_inc(s_pr, 1)

            # ---- BN tail ----
            v_.wait_ge(s_ev, NPASS)
            v_.tensor_reduce(s1p[:], s1cols[:], mybir.AxisListType.X,
                             Op.add).then_inc(s_dv, 1)
            v_.wait_ge(s_sq, NPASS)
            v_.tensor_reduce(s2p[:], s2cols[:], mybir.AxisListType.X,
                             Op.add).then_inc(s_dv, 1)
            v_.wait_ge(s_ac, 2)
            vcnt = 0

            def vstep(inst):
                nonlocal vcnt
                vcnt += 1
                inst.then_inc(s_vc, 1)
                v_.wait_ge(s_vc, vcnt)

            vstep(v_.tensor_tensor(msq[:], mean8[:], mean8[:], Op.mult))
            vstep(v_.tensor_tensor(var8[:], ey28[:], msq[:], Op.subtract))
            v_.tensor_scalar_add(var8[:], var8[:], BN_EPS).then_inc(s_dv, 1)
            v_.wait_ge(s_ac, 3)
            vstep(v_.reciprocal(rt[:], sqt[:]))
            for _i in range(2):
                vstep(v_.tensor_tensor(ut[:], rt[:], rt[:], Op.mult))
                vstep(v_.tensor_tensor(ut[:], ut[:], var8[:], Op.mult))
                vstep(v_.tensor_scalar(ut[:], ut[:], -0.5, 1.5,
                                       Op.mult, Op.add))
                vstep(v_.tensor_tensor(rt[:], rt[:], ut[:], Op.mult))
            vstep(v_.tensor_tensor(scsh8[:, 0:1], gma, rt[:], Op.mult))
            vstep(v_.tensor_tensor(scsh8[:, 1:2], mean8[:], scsh8[:, 0:1],
                                   Op.mult))
            v_.tensor_tensor(scsh8[:, 1:2], bta, scsh8[:, 1:2],
                             Op.subtract).then_inc(s_dv, 1)
            # affine chunks (one pass each) in the freed tmp buffers
            v_.wait_ge(s_bn, 32)
            for cch in range(NPASS):
                v_.wait_ge(s_yin, 16 * (cch + 1))
                v_.tensor_scalar(
                    tmps[cch % 2][:], tmps[cch % 2][:],
                    AB16[:, 0:1], AB16[:, 1:2],
                    Op.mult, Op.add).then_inc(s_p3, 1)

        # ---------------- PE: reduction matmuls ----------------
        @block.tensor
        def _(t_):
            t_.wait_ge(s_dmac, 32)
            for n in range(N):
                for h in range(2):
                    P = 2 * n + h
                    ab = abase(P)
                    if P >= 2:
                        t_.wait_ge(s_ev, P - 1)
                    if h == 0:
                        t_.wait_ge(s_dmaxs[n % 2], 16 * (n // 2 + 1))
                    # residual: start=True zeroes this pass's banks
                    for rq in range(2):
                        for cb in range(4):
                            t_.matmul(
                                acc48[ab:ab + 16, 4 * rq + cb, :],
                                selmm_sb[:, SEL_RES, :],
                                xpads[n % 2][:, 1 + 32 * h + 16 * rq + 4 * cb:
                                             1 + 32 * h + 16 * rq + 4 * cb + 4,
                                             1:129],
                                start=True, stop=False,
                                skip_group_check=True)

                    def unit(tile, sel, colsl, sem, is_last):
                        for rq in range(2):
                            for cb in range(4):
                                mm = t_.matmul(
                                    acc48[ab:ab + 16, 4 * rq + cb, :],
                                    selmm_sb[:, sel, :],
                                    tile[:, 16 * rq + 4 * cb:
                                         16 * rq + 4 * cb + 4, colsl],
                                    start=False,
                                    stop=is_last and rq == 1 and cb == 3,
                                    skip_group_check=True)
                                if rq == 1 and cb == 3:
                                    mm.then_inc(sem, 1)

                    # box passes: -x over the 6 min taps via xv
                    t_.wait_ge(s_xv, P + 1)
                    unit(xvs[P % 2], SEL_BOX, slice(0, 128), s_xvc, False)
                    unit(xvs[P % 2], SEL_BOX, slice(1, 129), s_xvc, False)
                    unit(xvs[P % 2], SEL_BOX, slice(2, 130), s_xvc, False)
                    for j in range(CP):
                        st = P * 8 + j
                        last_j = (j == CP - 1)
                        for k in range(2):  # pairs
                            pidx = st * 2 + k
                            t_.wait_ge(s_pr, pidx + 1)
                            unit(RP[pidx % RP_N], SEL_PAIR0 + j,
                                 slice(k, k + 128), s_pg, False)
                        for i in range(2):  # DVE singles
                            t_.wait_ge(s_Td, st * 6 + 5 + i)
                            sidx = st * 2 + i
                            unit(RS[sidx % RS_N], SEL_PAIR0 + j,
                                 slice(2, 130), s_ms, False)
                        for k in range(3):  # ACT tiles
                            aidx = st * 3 + k
                            t_.wait_ge(s_Ta, aidx + 1)
                            unit(RA[aidx % RA_N], SEL_ABS0 + j,
                                 slice(k, k + 128), s_sg, last_j and k == 2)

            # BN folds: [16] -> [8] selection matmuls
            t_.wait_ge(s_dv, 1)
            t_.matmul(acc48[0:8, 0, 0:1], c32[0:16, COL_S1F:COL_S1F + 8],
                      s1p[:], start=True, stop=True,
                      skip_group_check=True).then_inc(s_pe, 1)
            t_.wait_ge(s_dv, 2)
            t_.matmul(acc48[0:8, 1, 0:1], c32[0:16, COL_S1F:COL_S1F + 8],
                      s2p[:], start=True, stop=True,
                      skip_group_check=True).then_inc(s_pe, 1)

        # ---------------- ACT: abs tiles + evac + BN tail -------------
        @block.scalar
        def _(a):
            a.wait_ge(s_dmac, 32)

            def evac(P):
                # pass P done when its last ACT-cluster matmul retires
                # (s_sg hits 24*(P+1)); matmul then_inc is PSUM-commit-safe
                a.wait_ge(s_sg, 24 * (P + 1))
                if P >= 2:
                    a.wait_ge(s_ydma, 16 * (P - 1))
                ab = abase(P)
                a.activation(
                    tmps[P % 2][:],
                    acc48[ab:ab + 16, :, :].rearrange("p a b -> p (a b)"),
                    AF.Identity, bias=cOFF, scale=1.0,
                    accum_out=s1cols[:, P:P + 1]).then_inc(s_ev, 1)
                # self-sync: the square and the DMA read tmp the evac just
                # wrote; wait for the evac write to retire first
                a.wait_ge(s_ev, P + 1)
                # square scratch = the other tmp (its ybuf store is done)
                if P >= 1:
                    a.wait_ge(s_ydma, 16 * P)
                a.activation(
                    tmps[(P + 1) % 2][:], tmps[P % 2][:], AF.Square,
                    accum_out=s2cols[:, P:P + 1]).then_inc(s_sq, 1)
                a.wait_ge(s_sq, P + 1)
                a.dma_start(ybuf[:, P, :],
                            tmps[P % 2][:]).then_inc(s_ydma, 16)

            for n in range(N):
                for h in range(2):
                    P = 2 * n + h
                    if h == 0:
                        a.wait_ge(s_dmaxs[n % 2], 16 * (n // 2 + 1))
                    for j in range(CP):
                        st = P * 8 + j
                        for k in range(3):  # taps (2,0),(2,1),(2,2)
                            aidx = st * 3 + k
                            if aidx - (RA_N - 1) > 0:
                                a.wait_ge(s_sg, aidx - (RA_N - 1))
                            a.activation(
                                RA[aidx % RA_N][:, :, 0:131],
                                src(n, h, 2)[:, :, 0:131], AF.Abs,
                                bias=wcol(j, 6 + k),
                                scale=-1.0).then_inc(s_Ta, 1)
                        if j == 1 and P >= 1:
                            evac(P - 1)
            evac(NPASS - 1)

            # ---- BN tail ----
            a.wait_ge(s_pe, 1)
            a.mul(mean8[:], acc48[0:8, 0, 0:1], 1.0 / CNT).then_inc(s_ac, 1)
            a.wait_ge(s_pe, 2)
            a.mul(ey28[:], acc48[0:8, 1, 0:1], 1.0 / CNT).then_inc(s_ac, 1)
            a.wait_ge(s_dv, 3)
            a.activation(sqt[:], var8[:], AF.Sqrt).then_inc(s_ac, 1)
            a.wait_ge(s_dv, 4)
            a.dma_start(bnscr[0:1, 0:16], scsh8[:]).then_inc(s_bn, 16)
            a.wait_ge(s_bn, 16)
            # AB16[2j+b] = (A_j, B_j)
            a.dma_start(
                AB16[:],
                bnscr[0:1, 0:16].rearrange("a (j e) -> a j e", e=2)
                .unsqueeze(2).broadcast_to([1, 8, 2, 2])).then_inc(s_bn, 16)
            # affine: load ybuf chunk -> DVE affine -> store to out
            # (interleaved: the in-order ACT queue must not emit a load
            # whose wait depends on a store emitted later)
            a.wait_ge(s_ydma, 16 * NPASS)

            def outdma(cch):
                nn, hh = cch // 2, cch % 2
                dst = out[8 * nn:8 * nn + 8, :].rearrange(
                    "p (b h2 r c) -> p b h2 (r c)", b=2, h2=2, r=32
                )[:, :, hh, :]
                a.dma_start(dst, tmps[cch % 2][:]).then_inc(s_bn, 16)

            for cch in range(NPASS):
                if cch >= 2:
                    a.wait_ge(s_bn, 32 + 16 * (cch - 1))
                a.dma_start(tmps[cch % 2][:],
                            ybuf[:, cch, :]).then_inc(s_yin, 16)
                if cch >= 1:
                    a.wait_ge(s_p3, cch)
                    outdma(cch - 1)
            a.wait_ge(s_p3, NPASS)
            outdma(NPASS - 1)
            a.wait_ge(s_bn, 32 + 16 * NPASS)

    return nc


_LAST_RESULTS = None


def _host_inputs(x, weight, gamma, beta):
    x = np.ascontiguousarray(np.asarray(x, dtype=np.float32))
    weight = np.asarray(weight, dtype=np.float32)
    gamma = np.asarray(gamma, dtype=np.float32)
    beta = np.asarray(beta, dtype=np.float32)

    x16 = x.astype(np.float16)
    x16p = np.zeros((N, 128, ROWS, RW), np.float16)
    x16p[:, 0:64, 1:66, 1:129] = x16[:, :, 0:65, :]
    x16p[:, 64:128, 0:65, 1:129] = x16[:, :, 63:128, :]
    x16p = x16p.reshape(N, 128, ROWS * RW)

    in_maps = []
    for core in range(NCORES):
        cs = slice(CP * core, CP * (core + 1))
        wslice = weight[cs]
        warr = np.tile(
            wslice.transpose(1, 0, 2, 3).reshape(64, CP * 9), (2, 1)
        ).astype(np.float32)
        c32 = np.zeros((128, NC32), np.float32)
        c32[:, 0:CP * 9] = warr
        c32[0:8, COL_G] = gamma[cs]
        c32[0:8, COL_B] = beta[cs]
        c32[np.arange(16), COL_S1F + np.arange(16) // 2] = 1.0
        for j in range(CP):
            wf = wslice[j].reshape(64, 9).astype(np.float64)
            e_abs = 0.0
            for ci in range(64):
                for t in range(9):
                    wv = float(wf[ci, t])
                    e_abs += (math.sqrt(2.0 / math.pi)
                              * math.exp(-0.5 * wv * wv)
                              + wv * math.erf(wv / math.sqrt(2.0)))
            # evac bias: E[sum|x-w|] + sum_{min taps} w centers fp16 range
            coff = e_abs + float(wf[:, 0:6].sum())
            c32[2 * j, COL_OFF] = coff
            c32[2 * j + 1, COL_OFF] = coff

        selmm = np.zeros((128, NSEL, 16), np.float16)
        for b in range(2):
            rows = slice(b * 64, (b + 1) * 64)
            for j in range(CP):
                selmm[rows, SEL_PAIR0 + j, 2 * j + b] = 2.0
                selmm[rows, SEL_ABS0 + j, 2 * j + b] = -1.0
            selmm[rows, SEL_BOX, b::2] = -1.0
        for j in range(CP):
            cog = CP * core + j
            for b in range(2):
                selmm[b * 64 + cog, SEL_RES, 2 * j + b] = 1.0
        in_maps.append({
            "x16p": x16p,
            "consts32": c32,
            "selmm": selmm,
        })
    return in_maps


def kernel(x, weight, gamma, beta, alpha):
    from concourse.bass_utils import run_bass_kernel_spmd

    nc = _build_program()
    in_maps = _host_inputs(x, weight, gamma, beta)

    trace = os.environ.get("ADDER_TRACE", "0") == "1"
    if os.environ.get("ADDER_WARMUP", "1") == "1":
        try:
            run_bass_kernel_spmd(nc, in_maps, core_ids=list(range(NCORES)),
                                 trace=False)
        except Exception:
            pass
    res = run_bass_kernel_spmd(nc, in_maps, core_ids=list(range(NCORES)),
                               trace=trace)
    global _LAST_RESULTS
    _LAST_RESULTS = res

    # out rows 8n+j; pixel order (b, h, r, c) row-major = plain (h, w)
    parts = [r["out"].astype(np.float32).reshape(N, CP, H, W)
             for r in res.results]
    full = np.concatenate(parts, axis=1).astype(np.float32)

    a = float(np.asarray(alpha))
    if a != 1.0:
        full = np.sign(full) * np.power(np.abs(full) + 1e-12, a,
                                        dtype=np.float32)
    return full


# revision 34
# speedup vs baseline: 1.2174x; 1.0120x over previous
"""AdderNet layer (adder2d + residual + BatchNorm(train) + PowerActivation)
on 8 Trainium2 NeuronCores. Raw Bass, explicit semaphores.

v4: all-abs production via the abs_max ALU trick (|x-w| = abs_max(x-w, 0)
in ONE DVE tensor_scalar at the 4x perf mode), giving every tap tile a
uniform PE coefficient of -1. This removes the v3 box-sum passes and the
min/abs algebra split. 32-row tiles (2x fewer elementwise ops than v3's
16-row groups -> amortized per-op overhead), PSUM laid out as
[16 outs, 8 banks] per 32-row pass (two pass buffers at partition bases
0/32 of one [48,8,512] tensor), so evac is ONE activation per pass, and
Sum(y^2) is accumulated during evac (kills the separate variance pass
and the mean64 broadcast). The fp16 y buffer lives in DRAM (frees
~30KB/partition of SBUF); the BN affine tail streams it back through
the freed tmp buffers.

Per (j, 32-row pass): 6 DVE abs tiles + 3 ACT abs tiles; DVE pre-merges
the vertical pairs (kh0,kw)+(kh1,kw) for kw=0,1 -> 7 PE clusters of 8
matmuls each + shared residual. Engine busy model per pass: DVE ~95us,
ACT ~97us, PE ~97us - balanced within ~3%.

Self-contained: hardcodes N,C,H,W=8,64,128,128, CO=64, K=3, pad=1.
Sharding by OUTPUT CHANNEL (8 co per core): BN stats core-local.
PowerActivation alpha=1.0 is identity (harness value); exact host
fallback otherwise. kernel() runs a warmup execution first.
"""

import math
import os
from contextlib import ExitStack

import numpy as np

N, C, H, W = 8, 64, 128, 128
CO, KS = 64, 3
BN_EPS = 1e-5
NCORES = 8
CP = CO // NCORES
RW = 132
ROWS = 66
PIX = H * W
CNT = float(N * PIX)
NPASS = 2 * N            # 16 32-row passes

# ring sizes (tiles of [128, 32, <=130] fp16, ~8.3 KB/partition)
RD_N = 4    # DVE pair-feed tiles (consumed by DVE merges, same queue)
RP_N = 4    # merged pair tiles -> PE
RS_N = 4    # DVE single tiles (kw=2) -> PE
RA_N = 4    # ACT abs tiles -> PE

SEL_PAIR0 = 0             # selmm slices: 0..7 pair/min lhs (+2) per j
SEL_ABS0 = 8              # 8..15 abs lhs (-1) per j
SEL_BOX = 16              # -1 box-sum, all j columns
SEL_RES = 17              # +1 residual
NSEL = 18

COL_G = 72
COL_B = 73
COL_OFF = 74
COL_S1F = 76              # sel: [2j+b, j] = 1 (rows 0:16, 8 cols)
NC32 = 92

TS_PER_PASS = 6 * CP      # DVE tensor_scalar productions per 32-row pass
TA_PER_PASS = 3 * CP      # ACT productions per pass


def _build_program():
    import concourse.bass as bass
    import concourse.mybir as mybir
    from concourse.mybir import AluOpType as Op

    f32 = mybir.dt.float32
    f16 = mybir.dt.float16
    AF = mybir.ActivationFunctionType

    nc = bass.Bass("TRN2")

    x16p = nc.dram_tensor("x16p", [N, 128, ROWS * RW], f16,
                          kind="ExternalInput")
    consts32 = nc.dram_tensor("consts32", [128, NC32], f32,
                              kind="ExternalInput")
    selmm = nc.dram_tensor("selmm", [128, NSEL, 16], f16,
                           kind="ExternalInput")
    out = nc.dram_tensor("out", [64, PIX], f16, kind="ExternalOutput")
    ybuf = nc.dram_tensor("ybuf", [16, NPASS, 4096], f16, kind="Internal")
    bnscr = nc.dram_tensor("bnscr", [1, 16], f32, kind="Internal")

    ctx = ExitStack()
    with ctx:
        c32 = ctx.enter_context(nc.sbuf_tensor("c32", [128, NC32], f32))
        selmm_sb = ctx.enter_context(
            nc.sbuf_tensor("selmm_sb", [128, NSEL, 16], f16))
        xpad0 = ctx.enter_context(nc.sbuf_tensor("xpad0", [128, ROWS, RW], f16))
        xpad1 = ctx.enter_context(nc.sbuf_tensor("xpad1", [128, ROWS, RW], f16))
        xpads = [xpad0, xpad1]
        RD = [ctx.enter_context(nc.sbuf_tensor(f"RD{i}", [128, 32, 132], f16))
              for i in range(RD_N)]
        RP = [ctx.enter_context(nc.sbuf_tensor(f"RP{i}", [128, 32, 132], f16))
              for i in range(RP_N)]
        RS = [ctx.enter_context(nc.sbuf_tensor(f"RS{i}", [128, 32, 132], f16))
              for i in range(RS_N)]
        RA = [ctx.enter_context(nc.sbuf_tensor(f"RA{i}", [128, 32, 132], f16))
              for i in range(RA_N)]
        xvs = [ctx.enter_context(nc.sbuf_tensor(f"xv{i}", [128, 32, 132], f16))
               for i in range(2)]
        tmp0 = ctx.enter_context(nc.sbuf_tensor("tmp0", [16, 4096], f16))
        tmp1 = ctx.enter_context(nc.sbuf_tensor("tmp1", [16, 4096], f16))
        tmps = [tmp0, tmp1]

        s1cols = ctx.enter_context(nc.sbuf_tensor("s1cols", [16, NPASS], f32))
        s2cols = ctx.enter_context(nc.sbuf_tensor("s2cols", [16, NPASS], f32))
        s1p = ctx.enter_context(nc.sbuf_tensor("s1p", [16, 1], f32))
        s2p = ctx.enter_context(nc.sbuf_tensor("s2p", [16, 1], f32))
        mean8 = ctx.enter_context(nc.sbuf_tensor("mean8", [8, 1], f32))
        ey28 = ctx.enter_context(nc.sbuf_tensor("ey28", [8, 1], f32))
        msq = ctx.enter_context(nc.sbuf_tensor("msq", [8, 1], f32))
        var8 = ctx.enter_context(nc.sbuf_tensor("var8", [8, 1], f32))
        sqt = ctx.enter_context(nc.sbuf_tensor("sqt", [8, 1], f32))
        rt = ctx.enter_context(nc.sbuf_tensor("rt", [8, 1], f32))
        ut = ctx.enter_context(nc.sbuf_tensor("ut", [8, 1], f32))
        scsh8 = ctx.enter_context(nc.sbuf_tensor("scsh8", [8, 2], f32))
        AB16 = ctx.enter_context(nc.sbuf_tensor("AB16", [16, 2], f32))

        acc48 = ctx.enter_context(nc.psum_tensor("acc48", [48, 8, 512], f32))

        s_dmac = ctx.enter_context(nc.semaphore())
        s_dmax0 = ctx.enter_context(nc.semaphore())
        s_dmax1 = ctx.enter_context(nc.semaphore())
        s_dmaxs = [s_dmax0, s_dmax1]
        s_Td = ctx.enter_context(nc.semaphore())   # DVE TS productions
        s_Ta = ctx.enter_context(nc.semaphore())   # ACT productions
        s_pr = ctx.enter_context(nc.semaphore())   # DVE pair merges
        s_pg = ctx.enter_context(nc.semaphore())   # PE consumed pair
        s_ms = ctx.enter_context(nc.semaphore())   # PE consumed RS single
        s_sg = ctx.enter_context(nc.semaphore())   # PE consumed ACT tile
        s_ev = ctx.enter_context(nc.semaphore())   # evac done (per pass)
        s_sq = ctx.enter_context(nc.semaphore())   # square done (per pass)
        s_ev2 = ctx.enter_context(nc.semaphore())  # PE pass drain
        s_ydma = ctx.enter_context(nc.semaphore())  # ybuf store done
        s_pe = ctx.enter_context(nc.semaphore())   # PE BN folds
        s_dv = ctx.enter_context(nc.semaphore())   # DVE BN steps
        s_ac = ctx.enter_context(nc.semaphore())   # ACT BN steps
        s_vc = ctx.enter_context(nc.semaphore())   # DVE serial chain
        s_bn = ctx.enter_context(nc.semaphore())   # BN dma chain
        s_yin = ctx.enter_context(nc.semaphore())  # affine chunk loads
        s_p3 = ctx.enter_context(nc.semaphore())   # affine chunks done
        s_xv = ctx.enter_context(nc.semaphore())   # xv tile produced
        s_x2 = ctx.enter_context(nc.semaphore())   # j0 singles pre-merge
        s_xvc = ctx.enter_context(nc.semaphore())  # PE consumed xv
        block = ctx.enter_context(nc.Block())

        gma = c32[0:8, COL_G:COL_G + 1]
        bta = c32[0:8, COL_B:COL_B + 1]
        cOFF = c32[0:16, COL_OFF:COL_OFF + 1]

        def wcol(j, t):
            return c32[:, j * 9 + t:j * 9 + t + 1]

        def src(n, h, kh):
            base = 32 * h + kh
            return xpads[n % 2][:, base:base + 32, :]

        def abase(P):
            return 32 * (P % 2)

        # global stage index: stage = P * 8 + j, P = 2n + h
        # ---------------- Pool: DMA loads ----------------
        @block.gpsimd
        def _(gp):
            gp.dma_start(c32[:], consts32[:]).then_inc(s_dmac, 16)
            gp.dma_start(selmm_sb[:], selmm[:]).then_inc(s_dmac, 16)
            for n in range(2):
                gp.dma_start(
                    xpads[n][:].rearrange("p r c -> p (r c)"),
                    x16p[n, :, :]).then_inc(s_dmaxs[n], 16)
            for nl in range(2, N):
                # image nl-2's readers must be done before overwrite
                gp.wait_ge(s_sg, 24 * 2 * (nl - 1))
                gp.wait_ge(s_Td, TS_PER_PASS * 2 * (nl - 1))
                gp.wait_ge(s_Ta, TA_PER_PASS * 2 * (nl - 1))
                gp.dma_start(
                    xpads[nl % 2][:].rearrange("p r c -> p (r c)"),
                    x16p[nl, :, :]).then_inc(s_dmaxs[nl % 2], 16)

        # ---------------- DVE: abs tiles + pair merges + BN ----------
        @block.vector
        def _(v_):
            v_.wait_ge(s_dmac, 32)
            for n in range(N):
                for h in range(2):
                    P = 2 * n + h
                    if h == 0:
                        v_.wait_ge(s_dmaxs[n % 2], 16 * (n // 2 + 1))
                    # xv: row-pair sum for the box passes
                    if P >= 2:
                        v_.wait_ge(s_xvc, 3 * (P - 1))
                    v_.tensor_tensor(
                        xvs[P % 2][:, :, 0:131],
                        src(n, h, 0)[:, :, 0:131],
                        src(n, h, 1)[:, :, 0:131],
                        Op.add).then_inc(s_xv, 1)
                    for j in range(CP):
                        st = P * 8 + j
                        # pair feeds: taps (0,0),(0,1),(1,0),(1,1)
                        for mi, (kh, kw) in enumerate(
                                ((0, 0), (0, 1), (1, 0), (1, 1))):
                            fidx = st * 4 + mi
                            v_.tensor_scalar(
                                RD[fidx % RD_N][:, :, 0:130],
                                src(n, h, kh)[:, :, 0:130],
                                wcol(j, 3 * kh + kw), 0.0,
                                Op.subtract, Op.min).then_inc(s_Td, 1)
                        if j == 1:  # pre-merge j0's singles (in-place)
                            st0 = P * 8
                            v_.tensor_tensor(
                                RS[(st0 * 2) % RS_N][:, :, 2:130],
                                RS[(st0 * 2) % RS_N][:, :, 2:130],
                                RS[(st0 * 2 + 1) % RS_N][:, :, 2:130],
                                Op.add).then_inc(s_x2, 1)
                        # singles: taps (0,2),(1,2) — emitted between the
                        # feeds and the merges so the merge reads are >2us
                        # behind the feed writes (same-engine RAW pipeline)
                        for i in range(2):
                            sidx = st * 2 + i
                            if sidx - (RS_N - 1) > 0:
                                v_.wait_ge(s_ms, sidx - (RS_N - 1))
                            v_.tensor_scalar(
                                RS[sidx % RS_N][:, :, 2:130],
                                src(n, h, i)[:, :, 2:130],
                                wcol(j, 3 * i + 2), 0.0,
                                Op.subtract, Op.min).then_inc(s_Td, 1)
                        for k in range(2):  # merge (0,k)+(1,k)
                            pidx = st * 2 + k
                            if pidx - (RP_N - 1) > 0:
                                v_.wait_ge(s_pg, pidx - (RP_N - 1))
                            fa = st * 4 + k
                            fb = st * 4 + 2 + k
                            v_.tensor_tensor(
                                RP[pidx % RP_N][:, :, 0:130],
                                RD[fa % RD_N][:, :, 0:130],
                                RD[fb % RD_N][:, :, 0:130],
                                Op.add).then_inc(s_pr, 1)

            # ---- BN tail ----
            v_.wait_ge(s_ev, NPASS)
            v_.tensor_reduce(s1p[:], s1cols[:], mybir.AxisListType.X,
                             Op.add).then_inc(s_dv, 1)
            v_.wait_ge(s_sq, NPASS)
            v_.tensor_reduce(s2p[:], s2cols[:], mybir.AxisListType.X,
                             Op.add).then_inc(s_dv, 1)
            v_.wait_ge(s_ac, 2)
            vcnt = 0

            def vstep(inst):
                nonlocal vcnt
                vcnt += 1
                inst.then_inc(s_vc, 1)
                v_.wait_ge(s_vc, vcnt)

            vstep(v_.tensor_tensor(msq[:], mean8[:], mean8[:], Op.mult))
            vstep(v_.tensor_tensor(var8[:], ey28[:], msq[:], Op.subtract))
            v_.tensor_scalar_add(var8[:], var8[:], BN_EPS).then_inc(s_dv, 1)
            v_.wait_ge(s_ac, 3)
            vstep(v_.reciprocal(rt[:], sqt[:]))
            for _i in range(2):
                vstep(v_.tensor_tensor(ut[:], rt[:], rt[:], Op.mult))
                vstep(v_.tensor_tensor(ut[:], ut[:], var8[:], Op.mult))
                vstep(v_.tensor_scalar(ut[:], ut[:], -0.5, 1.5,
                                       Op.mult, Op.add))
                vstep(v_.tensor_tensor(rt[:], rt[:], ut[:], Op.mult))
            vstep(v_.tensor_tensor(scsh8[:, 0:1], gma, rt[:], Op.mult))
            vstep(v_.tensor_tensor(scsh8[:, 1:2], mean8[:], scsh8[:, 0:1],
                                   Op.mult))
            v_.tensor_tensor(scsh8[:, 1:2], bta, scsh8[:, 1:2],
                             Op.subtract).then_inc(s_dv, 1)
            # affine chunks (one pass each) in the freed tmp buffers
            v_.wait_ge(s_bn, 32)
            for cch in range(NPASS):
                v_.wait_ge(s_yin, 16 * (cch + 1))
                v_.tensor_scalar(
                    tmps[cch % 2][:], tmps[cch % 2][:],
                    AB16[:, 0:1], AB16[:, 1:2],
                    Op.mult, Op.add).then_inc(s_p3, 1)

        # ---------------- PE: reduction matmuls ----------------
        @block.tensor
        def _(t_):
            t_.wait_ge(s_dmac, 32)
            for n in range(N):
                for h in range(2):
                    P = 2 * n + h
                    ab = abase(P)
                    if P >= 2:
                        t_.wait_ge(s_ev, P - 1)
                    if h == 0:
                        t_.wait_ge(s_dmaxs[n % 2], 16 * (n // 2 + 1))
                    # residual: start=True zeroes this pass's banks
                    for rq in range(2):
                        for cb in range(4):
                            t_.matmul(
                                acc48[ab:ab + 16, 4 * rq + cb, :],
                                selmm_sb[:, SEL_RES, :],
                                xpads[n % 2][:, 1 + 32 * h + 16 * rq + 4 * cb:
                                             1 + 32 * h + 16 * rq + 4 * cb + 4,
                                             1:129],
                                start=True, stop=False,
                                skip_group_check=True)

                    def unit(tile, sel, colsl, sem, is_last, inc=1):
                        for rq in range(2):
                            for cb in range(4):
                                mm = t_.matmul(
                                    acc48[ab:ab + 16, 4 * rq + cb, :],
                                    selmm_sb[:, sel, :],
                                    tile[:, 16 * rq + 4 * cb:
                                         16 * rq + 4 * cb + 4, colsl],
                                    start=False,
                                    stop=is_last and rq == 1 and cb == 3,
                                    skip_group_check=True)
                                if rq == 1 and cb == 3:
                                    mm.then_inc(sem, inc)

                    # box passes: -x over the 6 min taps via xv
                    t_.wait_ge(s_xv, P + 1)
                    unit(xvs[P % 2], SEL_BOX, slice(0, 128), s_xvc, False)
                    unit(xvs[P % 2], SEL_BOX, slice(1, 129), s_xvc, False)
                    unit(xvs[P % 2], SEL_BOX, slice(2, 130), s_xvc, False)
                    for j in range(CP):
                        st = P * 8 + j
                        last_j = (j == CP - 1)
                        for k in range(2):  # pairs
                            pidx = st * 2 + k
                            t_.wait_ge(s_pr, pidx + 1)
                            unit(RP[pidx % RP_N], SEL_PAIR0 + j,
                                 slice(k, k + 128), s_pg, False)
                        if j == 0:  # singles pre-merged on DVE
                            t_.wait_ge(s_x2, P + 1)
                            unit(RS[(st * 2) % RS_N], SEL_PAIR0 + j,
                                 slice(2, 130), s_ms, False, inc=2)
                        else:
                            for i in range(2):  # DVE singles
                                t_.wait_ge(s_Td, st * 6 + 5 + i)
                                sidx = st * 2 + i
                                unit(RS[sidx % RS_N], SEL_PAIR0 + j,
                                     slice(2, 130), s_ms, False)
                        for k in range(3):  # ACT tiles
                            aidx = st * 3 + k
                            t_.wait_ge(s_Ta, aidx + 1)
                            unit(RA[aidx % RA_N], SEL_ABS0 + j,
                                 slice(k, k + 128), s_sg, last_j and k == 2)

            # BN folds: [16] -> [8] selection matmuls
            t_.wait_ge(s_dv, 1)
            t_.matmul(acc48[0:8, 0, 0:1], c32[0:16, COL_S1F:COL_S1F + 8],
                      s1p[:], start=True, stop=True,
                      skip_group_check=True).then_inc(s_pe, 1)
            t_.wait_ge(s_dv, 2)
            t_.matmul(acc48[0:8, 1, 0:1], c32[0:16, COL_S1F:COL_S1F + 8],
                      s2p[:], start=True, stop=True,
                      skip_group_check=True).then_inc(s_pe, 1)

        # ---------------- ACT: abs tiles + evac + BN tail -------------
        @block.scalar
        def _(a):
            a.wait_ge(s_dmac, 32)

            def evac(P):
                # pass P done when its last ACT-cluster matmul retires
                # (s_sg hits 24*(P+1)); matmul then_inc is PSUM-commit-safe
                a.wait_ge(s_sg, 24 * (P + 1))
                if P >= 2:
                    a.wait_ge(s_ydma, 16 * (P - 1))
                ab = abase(P)
                a.activation(
                    tmps[P % 2][:],
                    acc48[ab:ab + 16, :, :].rearrange("p a b -> p (a b)"),
                    AF.Identity, bias=cOFF, scale=1.0,
                    accum_out=s1cols[:, P:P + 1]).then_inc(s_ev, 1)
                # self-sync: the square and the DMA read tmp the evac just
                # wrote; wait for the evac write to retire first
                a.wait_ge(s_ev, P + 1)
                # square scratch = the other tmp (its ybuf store is done)
                if P >= 1:
                    a.wait_ge(s_ydma, 16 * P)
                a.activation(
                    tmps[(P + 1) % 2][:], tmps[P % 2][:], AF.Square,
                    accum_out=s2cols[:, P:P + 1]).then_inc(s_sq, 1)
                a.wait_ge(s_sq, P + 1)
                a.dma_start(ybuf[:, P, :],
                            tmps[P % 2][:]).then_inc(s_ydma, 16)

            for n in range(N):
                for h in range(2):
                    P = 2 * n + h
                    if h == 0:
                        a.wait_ge(s_dmaxs[n % 2], 16 * (n // 2 + 1))
                    for j in range(CP):
                        st = P * 8 + j
                        for k in range(3):  # taps (2,0),(2,1),(2,2)
                            aidx = st * 3 + k
                            if aidx - (RA_N - 1) > 0:
                                a.wait_ge(s_sg, aidx - (RA_N - 1))
                            a.activation(
                                RA[aidx % RA_N][:, :, 0:131],
                                src(n, h, 2)[:, :, 0:131], AF.Abs,
                                bias=wcol(j, 6 + k),
                                scale=-1.0).then_inc(s_Ta, 1)
                        if j == 1 and P >= 1:
                            evac(P - 1)
            evac(NPASS - 1)

            # ---- BN tail ----
            a.wait_ge(s_pe, 1)
            a.mul(mean8[:], acc48[0:8, 0, 0:1], 1.0 / CNT).then_inc(s_ac, 1)
            a.wait_ge(s_pe, 2)
            a.mul(ey28[:], acc48[0:8, 1, 0:1], 1.0 / CNT).then_inc(s_ac, 1)
            a.wait_ge(s_dv, 3)
            a.activation(sqt[:], var8[:], AF.Sqrt).then_inc(s_ac, 1)
            a.wait_ge(s_dv, 4)
            a.dma_start(bnscr[0:1, 0:16], scsh8[:]).then_inc(s_bn, 16)
            a.wait_ge(s_bn, 16)
            # AB16[2j+b] = (A_j, B_j)
            a.dma_start(
                AB16[:],
                bnscr[0:1, 0:16].rearrange("a (j e) -> a j e", e=2)
                .unsqueeze(2).broadcast_to([1, 8, 2, 2])).then_inc(s_bn, 16)
            # affine: load ybuf chunk -> DVE affine -> store to out
            # (interleaved: the in-order ACT queue must not emit a load
            # whose wait depends on a store emitted later)
            a.wait_ge(s_ydma, 16 * NPASS)

            def outdma(cch):
                nn, hh = cch // 2, cch % 2
                dst = out[8 * nn:8 * nn + 8, :].rearrange(
                    "p (b h2 r c) -> p b h2 (r c)", b=2, h2=2, r=32
                )[:, :, hh, :]
                a.dma_start(dst, tmps[cch % 2][:]).then_inc(s_bn, 16)

            for cch in range(NPASS):
                if cch >= 2:
                    a.wait_ge(s_bn, 32 + 16 * (cch - 1))
                a.dma_start(tmps[cch % 2][:],
                            ybuf[:, cch, :]).then_inc(s_yin, 16)
                if cch >= 1:
                    a.wait_ge(s_p3, cch)
                    outdma(cch - 1)
            a.wait_ge(s_p3, NPASS)
            outdma(NPASS - 1)
            a.wait_ge(s_bn, 32 + 16 * NPASS)

    return nc


_LAST_RESULTS = None


def _host_inputs(x, weight, gamma, beta):
    x = np.ascontiguousarray(np.asarray(x, dtype=np.float32))
    weight = np.asarray(weight, dtype=np.float32)
    gamma = np.asarray(gamma, dtype=np.float32)
    beta = np.asarray(beta, dtype=np.float32)

    x16 = x.astype(np.float16)
    x16p = np.zeros((N, 128, ROWS, RW), np.float16)
    x16p[:, 0:64, 1:66, 1:129] = x16[:, :, 0:65, :]
    x16p[:, 64:128, 0:65, 1:129] = x16[:, :, 63:128, :]
    x16p = x16p.reshape(N, 128, ROWS * RW)

    in_maps = []
    for core in range(NCORES):
        cs = slice(CP * core, CP * (core + 1))
        wslice = weight[cs]
        warr = np.tile(
            wslice.transpose(1, 0, 2, 3).reshape(64, CP * 9), (2, 1)
        ).astype(np.float32)
        c32 = np.zeros((128, NC32), np.float32)
        c32[:, 0:CP * 9] = warr
        c32[0:8, COL_G] = gamma[cs]
        c32[0:8, COL_B] = beta[cs]
        c32[np.arange(16), COL_S1F + np.arange(16) // 2] = 1.0
        for j in range(CP):
            wf = wslice[j].reshape(64, 9).astype(np.float64)
            e_abs = 0.0
            for ci in range(64):
                for t in range(9):
                    wv = float(wf[ci, t])
                    e_abs += (math.sqrt(2.0 / math.pi)
                              * math.exp(-0.5 * wv * wv)
                              + wv * math.erf(wv / math.sqrt(2.0)))
            # evac bias: E[sum|x-w|] + sum_{min taps} w centers fp16 range
            coff = e_abs + float(wf[:, 0:6].sum())
            c32[2 * j, COL_OFF] = coff
            c32[2 * j + 1, COL_OFF] = coff

        selmm = np.zeros((128, NSEL, 16), np.float16)
        for b in range(2):
            rows = slice(b * 64, (b + 1) * 64)
            for j in range(CP):
                selmm[rows, SEL_PAIR0 + j, 2 * j + b] = 2.0
                selmm[rows, SEL_ABS0 + j, 2 * j + b] = -1.0
            selmm[rows, SEL_BOX, b::2] = -1.0
        for j in range(CP):
            cog = CP * core + j
            for b in range(2):
                selmm[b * 64 + cog, SEL_RES, 2 * j + b] = 1.0
        in_maps.append({
            "x16p": x16p,
            "consts32": c32,
            "selmm": selmm,
        })
    return in_maps


def kernel(x, weight, gamma, beta, alpha):
    from concourse.bass_utils import run_bass_kernel_spmd

    nc = _build_program()
    in_maps = _host_inputs(x, weight, gamma, beta)

    trace = os.environ.get("ADDER_TRACE", "0") == "1"
    if os.environ.get("ADDER_WARMUP", "1") == "1":
        try:
            run_bass_kernel_spmd(nc, in_maps, core_ids=list(range(NCORES)),
                                 trace=False)
        except Exception:
            pass
    res = run_bass_kernel_spmd(nc, in_maps, core_ids=list(range(NCORES)),
                               trace=trace)
    global _LAST_RESULTS
    _LAST_RESULTS = res

    # out rows 8n+j; pixel order (b, h, r, c) row-major = plain (h, w)
    parts = [r["out"].astype(np.float32).reshape(N, CP, H, W)
             for r in res.results]
    full = np.concatenate(parts, axis=1).astype(np.float32)

    a = float(np.asarray(alpha))
    if a != 1.0:
        full = np.sign(full) * np.power(np.abs(full) + 1e-12, a,
                                        dtype=np.float32)
    return full
